# revision 1
# baseline (speedup 1.0000x reference)
"""DenseGTVConv Trainium2 kernel (v2).

out = (I - (D - A~)) @ (x @ W) + bias,  A~ = adj / clamp(pairwise_L1(xW), 1e-3)

Per i-pair, an elementwise |dbl - S| op feeds a PE partition-reduction
matmul, so abs_diff lands in PSUM directly (no relu identity / S1/S2
correction).  Pairs are split across vector (bf16 tmp, sliding-E bf16
matmul) and ACT/gpsimd (fp8 tmp, DoubleRow fp8 matmul at 0.5 cyc/row).
ACT computes 1/(abs_diff + 1e-3) straight from PSUM; modbf = adj * recip;
transposed chunks feed the final (A~ @ xw) matmul whose rhs carries an
appended ones column so deg falls out of the same matmul.

Sharding: 8 cores = batch (2) x row-blocks (4 x 256 rows). Each core gets
the full x of its batch (needed on the j side), its 256-row slice of adj
(bf16, diag zeroed), and computes its 256-row slice of the output.

Self-contained: hardcoded shapes for B=2, N=1024, F_in=128, F_out=64.
"""
import sys

sys.path.insert(0, "/opt/trn_rl_repo")

from contextlib import ExitStack

import numpy as np
import ml_dtypes

import concourse.bass as bass
import concourse.bacc as bacc
import concourse.tile as tile
from concourse.masks import make_identity
from concourse import mybir
from concourse._compat import with_exitstack
from concourse.bass_utils import run_bass_kernel_spmd

F32 = mybir.dt.float32
BF16 = mybir.dt.bfloat16
FP8 = mybir.dt.float8e4

B, N, C, F = 2, 1024, 128, 64  # batch, nodes, f_in, f_out
R = 256  # rows per core
NCH = N // 128  # 8 column chunks of 128
NPAIR = R // 2  # 128 i-pairs per core
EPS = 1e-3

# Packed bf16 setup input [128, 1408]:
#   cols    0:1024 : xT      (x_b.T)
#   cols 1024:1280 : xrT     (x_rows.T)
#   cols 1280:1344 : W       [128, 64]
#   cols 1344:1408 : bias in partition 0, cols 0:64
XALL_COLS = N + R + 2 * F

# Hot-loop schedule per q: (kind, count) pairs; counts must sum to 64.
#   'ab' = ACT bf16 relu pair, 'vb' = vector bf16 relu pair. Rows are
#   assigned in listed order; all pairs feed M=128 sliding-E matmuls.
SCHED = [("ab", 20), ("vb", 44)]

ABS_MODE = "relu"  # relu identity: sum|d| = 2*sum(relu(d)) + S2[i] - S1[j]


def _expand_sched():
    """-> list of slot dicts; t = pair index within q (also row/2)."""
    slots = []
    t = 0
    for kind, cnt in SCHED:
        for _ in range(cnt):
            slots.append(dict(kind=kind, t=t))
            t += 1
    assert t == 64, f"pairs = {t}"
    return slots


SLOTS = _expand_sched()


def _pe_order(slots):
    """Weave ab pairs through the vb stream so the PE always has ready work:
    vb tiles appear at V's pace; ab tiles (produced concurrently on ACT)
    fill PE gaps. Uniform fractional merge, ab shifted slightly later."""
    vb = [s for s in slots if s["kind"] == "vb"]
    ab = [s for s in slots if s["kind"] == "ab"]
    if not ab or not vb:
        return slots
    keyed = [((i + 0.5) / len(vb), s) for i, s in enumerate(vb)]
    keyed += [((j + 1.5) / (len(ab) + 1), s) for j, s in enumerate(ab)]
    return [s for _, s in sorted(keyed, key=lambda p: p[0])]


def _act_recip(sc, out, in_, bias):
    """Scalar-engine Reciprocal(in + bias), bypassing the accuracy guard.
    Inputs here are in [35, 120] (pairwise L1 sums), far from the edge
    cases; the job tolerance is 2e-2 and the spline is ~1e-3-accurate."""
    inputs = [sc.lower_ap(in_)]
    for arg in (bias, 1.0, 0.0):  # bias, scale, alpha
        inputs.append(mybir.ImmediateValue(dtype=mybir.dt.float32, value=arg))
    return sc.add_instruction(
        mybir.InstActivation(
            name=sc.bass.get_next_instruction_name(),
            func=mybir.ActivationFunctionType.Reciprocal,
            ins=inputs,
            outs=[sc.lower_ap(out)],
        )
    )


@with_exitstack
def _body(ctx: ExitStack, tc: "tile.TileContext", io: dict):
    nc = tc.nc
    const = ctx.enter_context(tc.tile_pool(name="const", bufs=1))
    tmpv_pool = ctx.enter_context(tc.tile_pool(name="tmpv", bufs=10))
    tmp8a_pool = ctx.enter_context(tc.tile_pool(name="tmp8a", bufs=6))
    recip_pool = ctx.enter_context(tc.tile_pool(name="recip", bufs=2))
    modbf_pool = ctx.enter_context(tc.tile_pool(name="modbf", bufs=2))
    setup_ps = ctx.enter_context(tc.tile_pool(name="sps", bufs=2, space="PSUM"))
    ad_ps = ctx.enter_context(tc.tile_pool(name="adps", bufs=2, space="PSUM"))
    trfin_ps = ctx.enter_context(tc.tile_pool(name="trfin", bufs=2, space="PSUM"))

    # ---- input DMAs ----
    xallb = const.tile([128, XALL_COLS], BF16)
    nc.sync.dma_start(xallb[:, N:XALL_COLS], io["xallb"][:, N:XALL_COLS])
    nc.sync.dma_start(xallb[:, 0:512], io["xallb"][:, 0:512])
    nc.sync.dma_start(xallb[:, 512:N], io["xallb"][:, 512:N])
    adjq = []
    for q in range(2):
        a = const.tile([128, N], BF16, tag=f"adj{q}", name=f"adj{q}")
        nc.sync.dma_start(a[:], io["adjb"][128 * q : 128 * q + 128, :])
        adjq.append(a)

    xTb = xallb[:, 0:N]
    xrTb = xallb[:, N : N + R]
    w_sb = xallb[:, N + R : N + R + F]
    bias_sb = xallb[0:1, N + R + F : N + R + 2 * F]

    identb = const.tile([128, 128], BF16)
    make_identity(nc, identb[:])

    # ---- xwT -> dbl (bf16, f stacked twice on partitions) ----
    dbl = const.tile([128, N], BF16)
    for h in range(2):
        ps = setup_ps.tile([128, 512], F32, tag="sps", name="sps")
        nc.tensor.matmul(
            ps[0:64, :], w_sb, xTb[:, 512 * h : 512 * h + 512], start=True, stop=True
        )
        nc.vector.tensor_copy(dbl[0:64, 512 * h : 512 * h + 512], ps[0:64, :])
    nc.vector.tensor_copy(dbl[64:128, :], dbl[0:64, :])

    # ---- xwT_rows (exact i-side) -> per-pair scalars S (bf16) / negS (f32) ----
    xwT_rows = const.tile([64, R], F32)
    ps = setup_ps.tile([128, 512], F32, tag="sps", name="sps")
    nc.tensor.matmul(ps[0:64, 0:R], w_sb, xrTb[:], start=True, stop=True)
    nc.vector.tensor_copy(xwT_rows[:], ps[0:64, 0:R])

    S_bf = const.tile([128, NPAIR], F32)
    nc.vector.tensor_copy(S_bf[0:64, :], xwT_rows[:, 0:R:2])
    nc.vector.tensor_copy(S_bf[64:128, :], xwT_rows[:, 1:R:2])
    negS = const.tile([128, NPAIR], F32)
    nc.vector.tensor_scalar(negS[:], S_bf[:], -1.0, None, mybir.AluOpType.mult)

    if ABS_MODE == "bitwise":
        masku = const.tile([128, 1], mybir.dt.uint32)
        nc.vector.memset(masku[:], 0x7FFFFFFF)
        maskf = masku[:].bitcast(F32)
    else:
        # row/col sums for the relu identity: sum|d| = 2*sum(relu(d)) - S1[j] + S2[i]
        ones64b = const.tile([64, 1], BF16)
        nc.vector.memset(ones64b[:], 1.0)
        ones64f = const.tile([64, 1], F32)
        nc.vector.memset(ones64f[:], 1.0)
        ones1f = const.tile([1, 128], F32)
        nc.vector.memset(ones1f[:], 1.0)
        s1row = const.tile([1, N], F32)
        for h in range(2):
            ps = setup_ps.tile([128, 512], F32, tag="sps", name="sps")
            nc.tensor.matmul(
                ps[0:1, :], ones64b[:], dbl[0:64, 512 * h : 512 * h + 512],
                start=True, stop=True,
            )
            nc.scalar.copy(s1row[:, 512 * h : 512 * h + 512], ps[0:1, :])
        S1bc = const.tile([128, N], F32)
        for h in range(2):
            ps = setup_ps.tile([128, 512], F32, tag="sps", name="sps")
            nc.tensor.matmul(
                ps[:, :], ones1f[:], s1row[0:1, 512 * h : 512 * h + 512],
                start=True, stop=True,
            )
            nc.scalar.copy(S1bc[:, 512 * h : 512 * h + 512], ps[:, :])
        S2 = const.tile([128, 2], F32)
        for qq in range(2):
            ps = setup_ps.tile([128, 512], F32, tag="sps", name="sps")
            nc.tensor.matmul(
                ps[:, 0:1], xwT_rows[:, 128 * qq : 128 * qq + 128], ones64f[:],
                start=True, stop=True,
            )
            nc.scalar.copy(S2[:, qq : qq + 1], ps[:, 0:1])

    # ---- sliding reduction weights: Eb [128, 254], slice
    # [:, 126-2r : 254-2r] = ones at (p 0:64 -> row 2r), (p 64:128 -> 2r+1).
    Eb = const.tile([128, 254], BF16)
    nc.vector.memset(Eb[:], 0.0)
    nc.vector.memset(Eb[0:64, 126:127], 1.0)
    nc.vector.memset(Eb[64:128, 127:128], 1.0)

    # ---- xw (bf16, j on partitions per chunk) + ones col -> final rhs ----
    xwb1 = const.tile([128, NCH * (F + 1)], BF16)
    for c in range(NCH):
        ps = setup_ps.tile([128, 512], F32, tag="sps", name="sps")
        nc.tensor.matmul(
            ps[:, 0:F], xTb[:, 128 * c : 128 * c + 128], w_sb, start=True, stop=True
        )
        nc.vector.tensor_copy(xwb1[:, (F + 1) * c : (F + 1) * c + F], ps[:, 0:F])
    nc.vector.memset(xwb1[:, F : NCH * (F + 1) : F + 1], 1.0)

    xw_rows = const.tile([128, 2 * F], F32)
    for q in range(2):
        ps = setup_ps.tile([128, 512], F32, tag="sps", name="sps")
        nc.tensor.matmul(
            ps[:, 0:F], xrTb[:, 128 * q : 128 * q + 128], w_sb, start=True, stop=True
        )
        nc.vector.tensor_copy(xw_rows[:, F * q : F * q + F], ps[:, 0:F])

    # ---- bias broadcast [128, F] via K=1 matmul ----
    ones1 = const.tile([1, 128], BF16)
    nc.scalar.activation(
        ones1[:], xallb[0:1, 0:128], mybir.ActivationFunctionType.Copy,
        bias=1.0, scale=0.0,
    )
    bias_bc = const.tile([128, F], F32)
    ps = setup_ps.tile([128, 512], F32, tag="sps", name="sps")
    nc.tensor.matmul(ps[:, 0:F], ones1[:], bias_sb, start=True, stop=True)
    nc.vector.tensor_copy(bias_bc[:], ps[:, 0:F])

    modT = [
        const.tile([128, R], BF16, tag=f"modT{jc}", name=f"modT{jc}")
        for jc in range(NCH)
    ]
    out_sb = [const.tile([128, F], F32, tag=f"osb{q}", name=f"osb{q}") for q in range(2)]

    # ---- hot loop over q-blocks of 128 rows ----
    for q in range(2):
        adps = [
            ad_ps.tile([128, 512], F32, tag=f"adps{k}", name=f"adps{q}_{k}")
            for k in range(2)
        ]

        # 1) elementwise producers, per engine
        for s in SLOTS:
            t = 64 * q + s["t"]
            if s["kind"] == "vb":
                tmpb = tmpv_pool.tile([128, N], BF16, tag="tv", name="tv")
                nc.vector.tensor_scalar(
                    tmpb[:], dbl[:], S_bf[:, t : t + 1], 0.0,
                    mybir.AluOpType.subtract, mybir.AluOpType.max,
                )
                s["tile"] = tmpb
        for s in SLOTS:
            t = 64 * q + s["t"]
            if s["kind"] == "ab":
                tmpa = tmp8a_pool.tile([128, N], BF16, tag="ta", name="ta")
                nc.scalar.activation(
                    tmpa[:], dbl[:], mybir.ActivationFunctionType.Relu,
                    bias=negS[:, t : t + 1], scale=1.0,
                )
                s["tile"] = tmpa

        # 2) reduction matmuls, ab woven through vb; one accumulation
        # group per (q, k) over the full [128, 512] tile
        pe_order = _pe_order(SLOTS)
        for i, s in enumerate(pe_order):
            r = s["t"]
            for k in range(2):
                nc.tensor.matmul(
                    adps[k][:],
                    Eb[:, 126 - 2 * r : 254 - 2 * r],
                    s["tile"][:, 512 * k : 512 * k + 512],
                    start=(i == 0),
                    stop=(i == len(pe_order) - 1),
                )

        # 3) epilogue: recip = 1/(abs_diff + eps) on ACT
        recipbf = recip_pool.tile([128, N], BF16, tag="recip", name="recip")
        if ABS_MODE == "bitwise":
            for k in range(2):
                _act_recip(
                    nc.scalar, recipbf[:, 512 * k : 512 * k + 512], adps[k][:], EPS
                )
        else:
            ada = recip_pool.tile([128, N], F32, tag="ada", name="ada")
            for k in range(2):
                nc.vector.tensor_scalar(
                    ada[:, 512 * k : 512 * k + 512], adps[k][:], 2.0,
                    S2[:, q : q + 1], mybir.AluOpType.mult, mybir.AluOpType.add,
                )
            adf = recip_pool.tile([128, N], F32, tag="adf", name="adf")
            nc.gpsimd.tensor_tensor(adf[:], ada[:], S1bc[:], mybir.AluOpType.subtract)
            for k in range(2):
                _act_recip(
                    nc.scalar, recipbf[:, 512 * k : 512 * k + 512],
                    adf[:, 512 * k : 512 * k + 512], EPS,
                )
        if "dbg_recip" in io:
            nc.sync.dma_start(io["dbg_recip"][128 * q : 128 * q + 128, :], recipbf[:])
        modbf = modbf_pool.tile([128, N], BF16, tag="modbf", name="modbf")
        nc.gpsimd.tensor_tensor(
            modbf[:, 0:512], adjq[q][:, 0:512], recipbf[:, 0:512],
            mybir.AluOpType.mult,
        )
        nc.vector.tensor_tensor(
            modbf[:, 512:N], adjq[q][:, 512:N], recipbf[:, 512:N],
            mybir.AluOpType.mult,
        )
        if "dbg_mod" in io:
            nc.sync.dma_start(io["dbg_mod"][128 * q : 128 * q + 128, :], modbf[:])
        for jc in range(NCH):
            tr = trfin_ps.tile([128, 128], BF16, tag="trfin", name="tr")
            nc.tensor.transpose(tr[:], modbf[:, 128 * jc : 128 * jc + 128], identb[:])
            if jc % 2 == 0:
                nc.vector.tensor_copy(modT[jc][:, 128 * q : 128 * q + 128], tr[:])
            else:
                nc.scalar.copy(modT[jc][:, 128 * q : 128 * q + 128], tr[:])

        # 4) final: fin[:, 0:64] = A~ @ xw, fin[:, 64] = deg
        fin = trfin_ps.tile([128, 128], F32, tag="trfin", name=f"fin{q}")
        for jc in range(NCH):
            nc.tensor.matmul(
                fin[:, 0 : F + 1],
                modT[jc][:, 128 * q : 128 * q + 128],
                xwb1[:, (F + 1) * jc : (F + 1) * jc + F + 1],
                start=(jc == 0),
                stop=(jc == NCH - 1),
            )
        onemdeg = const.tile([128, 1], F32, tag=f"od{q}", name=f"od{q}")
        nc.vector.tensor_scalar(
            onemdeg[:], fin[:, F : F + 1], -1.0, 1.0,
            mybir.AluOpType.mult, mybir.AluOpType.add,
        )
        corr = const.tile([128, F], F32, tag=f"corr{q}", name=f"corr{q}")
        nc.vector.tensor_scalar(
            corr[:], xw_rows[:, F * q : F * q + F], onemdeg[:, 0:1], None,
            mybir.AluOpType.mult,
        )
        nc.vector.tensor_tensor(corr[:], corr[:], bias_bc[:], mybir.AluOpType.add)
        nc.vector.tensor_tensor(out_sb[q][:], corr[:], fin[:, 0:F], mybir.AluOpType.add)
        if "dbg_deg" in io:
            dsb = const.tile([128, 1], F32, tag=f"dsb{q}", name=f"dsb{q}")
            nc.vector.tensor_copy(dsb[:], fin[:, F : F + 1])
            nc.sync.dma_start(io["dbg_deg"][:, q : q + 1], dsb[:])
        nc.sync.dma_start(io["out_block"][128 * q : 128 * q + 128, :], out_sb[q][:])


_CACHE = {}


def _build(debug=False):
    key = ("nc", debug)
    if key in _CACHE:
        return _CACHE[key]
    nc = bacc.Bacc()
    io = {
        "xallb": nc.declare_dram_parameter("xallb", [C, XALL_COLS], BF16, isOutput=False),
        "adjb": nc.declare_dram_parameter("adjb", [R, N], BF16, isOutput=False),
        "out_block": nc.declare_dram_parameter("out_block", [R, F], F32, isOutput=True),
    }
    if debug:
        io["dbg_recip"] = nc.declare_dram_parameter("dbg_recip", [R, N], BF16, isOutput=True)
        io["dbg_mod"] = nc.declare_dram_parameter("dbg_mod", [R, N], BF16, isOutput=True)
        io["dbg_deg"] = nc.declare_dram_parameter("dbg_deg", [128, 2], F32, isOutput=True)
    with tile.TileContext(nc) as tc:
        _body(tc, io)
    nc.finalize()
    _CACHE[key] = nc
    return nc


def _make_in_maps(x, adj, weight, bias):
    in_maps = []
    for core in range(8):
        b, blk = core // 4, core % 4
        r0 = blk * R
        xallb = np.zeros((C, XALL_COLS), dtype=ml_dtypes.bfloat16)
        xallb[:, 0:N] = x[b].T.astype(ml_dtypes.bfloat16)
        xallb[:, N : N + R] = x[b, r0 : r0 + R].T.astype(ml_dtypes.bfloat16)
        xallb[:, N + R : N + R + F] = weight.astype(ml_dtypes.bfloat16)
        xallb[0, N + R + F : N + R + 2 * F] = bias.astype(ml_dtypes.bfloat16)
        adjb = np.ascontiguousarray(adj[b, r0 : r0 + R]).copy()
        # Zero the self-edge: diag(mod_adj) cancels analytically in
        # out = (I - D + A~) xw, so drop it to avoid 1/0 on the diagonal.
        adjb[np.arange(R), r0 + np.arange(R)] = 0.0
        in_maps.append({"xallb": xallb, "adjb": adjb.astype(ml_dtypes.bfloat16)})
    return in_maps


def run(x, adj, weight, bias, trace=False, debug=False):
    nc = _build(debug=debug)
    res = run_bass_kernel_spmd(
        nc, _make_in_maps(x, adj, weight, bias), list(range(8)), trace=trace
    )
    out = np.empty((B, N, F), dtype=np.float32)
    for core in range(8):
        b, blk = core // 4, core % 4
        out[b, blk * R : blk * R + R] = res.results[core]["out_block"]
    return out, res


def kernel(x, adj, weight, bias):
    x = np.asarray(x, dtype=np.float32)
    adj = np.asarray(adj, dtype=np.float32)
    weight = np.asarray(weight, dtype=np.float32)
    bias = np.asarray(bias, dtype=np.float32)
    out, _ = run(x, adj, weight, bias, trace=False)
    return out



# revision 4
# speedup vs baseline: 1.0560x; 1.0560x over previous
"""DenseGTVConv Trainium2 kernel (v3).

out = (I - (D - A~)) @ (x @ W) + bias,  A~ = adj / clamp(pairwise_L1(xW), 1e-3)

Per i-pair, an elementwise relu(dbl - S) op feeds a PE partition-reduction
matmul (sliding-E), accumulating sum(relu) into PSUM.  The relu-identity
corrections -S1[j] and +S2[i]+eps are folded into the SAME PSUM
accumulation group via two K=1 matmuls, so the scalar engine computes
recip = 1/(2*psum) straight from PSUM.  modbf = adj * recip; transposed
chunks feed the final (A~ @ xw) matmul whose rhs carries an appended ones
column so deg falls out of the same matmul.

v3 structural changes vs v2:
 - PE warmup matmuls on zeros during the startup DMA window (HAM un-throttle)
 - input DMAs spread across scalar/vector/gpsimd/sync queues, xT first
 - S1/S2/eps folded into PSUM accumulation (no ada/adf passes, no S1bc)
 - cross-q software pipelining: q1 producers are emitted before q0's
   epilogue on each engine; PE interleaves q0 transposes/final matmuls
   into q1's sliding stream so it never idles
 - modbf half + modT copies on gpsimd (otherwise idle)

Sharding: 8 cores = batch (2) x row-blocks (4 x 256 rows). Each core gets
the full x of its batch (needed on the j side), its 256-row slice of adj
(bf16, diag zeroed), and computes its 256-row slice of the output.

Self-contained: hardcoded shapes for B=2, N=1024, F_in=128, F_out=64.
"""
import sys

sys.path.insert(0, "/opt/trn_rl_repo")

from contextlib import ExitStack

import numpy as np
import ml_dtypes

import concourse.bass as bass
import concourse.bacc as bacc
import concourse.tile as tile
from concourse.masks import make_identity
from concourse import mybir
from concourse._compat import with_exitstack
from concourse.bass_utils import run_bass_kernel_spmd

F32 = mybir.dt.float32
BF16 = mybir.dt.bfloat16

B, N, C, F = 2, 1024, 128, 64  # batch, nodes, f_in, f_out
R = 256  # rows per core
NCH = N // 128  # 8 column chunks of 128
NPAIR = R // 2  # 128 i-pairs per core
EPS = 1e-3

# Packed bf16 setup input [128, 1408]:
#   cols    0:1024 : xT      (x_b.T)
#   cols 1024:1280 : xrT     (x_rows.T)
#   cols 1280:1344 : W       [128, 64]
#   cols 1344:1408 : bias in partition 0, cols 0:64
XALL_COLS = N + R + 2 * F

# Hot-loop schedule per q: (kind, count); counts sum to 64.
#   'ab' = ACT relu pair, 'vb' = vector relu pair.
SCHED = [("ab", 20), ("vb", 44)]

NWARM = 4  # PE warmup matmuls during the startup DMA window
V_PRE = 10  # q1 vb tiles emitted before q0's modbf_v
A_PRE = 10  # q1 ab tiles emitted before q0's recips
PE_PRE1 = 6  # q1 pairs before q0's transposes
PE_PRE2 = 7  # q1 pairs between q0's transposes and q0's final matmul


def _expand_sched():
    slots = []
    t = 0
    for kind, cnt in SCHED:
        for _ in range(cnt):
            slots.append(dict(kind=kind, t=t))
            t += 1
    assert t == 64, f"pairs = {t}"
    return slots


def _pe_order(slots):
    """Weave ab pairs through the vb stream so the PE always has ready work."""
    vb = [s for s in slots if s["kind"] == "vb"]
    ab = [s for s in slots if s["kind"] == "ab"]
    if not ab or not vb:
        return slots
    keyed = [((i + 0.5) / len(vb), s) for i, s in enumerate(vb)]
    keyed += [((j + 1.5) / (len(ab) + 1), s) for j, s in enumerate(ab)]
    return [s for _, s in sorted(keyed, key=lambda p: p[0])]


def _act_recip(sc, out, in_, bias, scale=1.0):
    """Scalar-engine Reciprocal(scale*in + bias), bypassing the accuracy
    guard.  Inputs here are in [35, 120] (pairwise L1 sums), far from the
    edge cases; the job tolerance is 2e-2 and the spline is ~1e-3-accurate."""
    inputs = [sc.lower_ap(in_)]
    for arg in (bias, scale, 0.0):  # bias, scale, alpha
        inputs.append(mybir.ImmediateValue(dtype=mybir.dt.float32, value=arg))
    return sc.add_instruction(
        mybir.InstActivation(
            name=sc.bass.get_next_instruction_name(),
            func=mybir.ActivationFunctionType.Reciprocal,
            ins=inputs,
            outs=[sc.lower_ap(out)],
        )
    )


@with_exitstack
def _body(ctx: ExitStack, tc: "tile.TileContext", io: dict):
    nc = tc.nc
    const = ctx.enter_context(tc.tile_pool(name="const", bufs=1))
    tmpv_pool = ctx.enter_context(tc.tile_pool(name="tmpv", bufs=12))
    tmp8a_pool = ctx.enter_context(tc.tile_pool(name="tmp8a", bufs=10))
    recip_pool = ctx.enter_context(tc.tile_pool(name="recip", bufs=2))
    modbf_pool = ctx.enter_context(tc.tile_pool(name="modbf", bufs=2))
    setup_ps = ctx.enter_context(tc.tile_pool(name="sps", bufs=2, space="PSUM"))
    ad_ps = ctx.enter_context(tc.tile_pool(name="adps", bufs=2, space="PSUM"))
    trfin_ps = ctx.enter_context(tc.tile_pool(name="trfin", bufs=2, space="PSUM"))

    # ---- tiles living in const pool ----
    xallb = const.tile([128, XALL_COLS], BF16)
    adjq = [
        const.tile([128, N], BF16, tag=f"adj{q}", name=f"adj{q}") for q in range(2)
    ]
    junk = const.tile([128, 512], BF16)

    # ---- V: junk memset first so PE warmup can start ASAP ----
    nc.vector.memset(junk[:], 0.0)

    # ---- input DMAs spread across engine queues, xT chunks first ----
    nc.scalar.dma_start(xallb[:, N:XALL_COLS], io["xallb"][:, N:XALL_COLS])
    nc.scalar.dma_start(xallb[:, 0:256], io["xallb"][:, 0:256])
    nc.gpsimd.dma_start(xallb[:, 256:512], io["xallb"][:, 256:512])
    nc.gpsimd.dma_start(xallb[:, 512:N], io["xallb"][:, 512:N])
    nc.sync.dma_start(adjq[0][:], io["adjb"][0:128, :])
    nc.sync.dma_start(adjq[1][:], io["adjb"][128:256, :])

    # ---- PE warmup: zeros matmuls keep the PE busy from t~0 so the HAM
    # clock gate un-throttles before real work arrives ----
    for w in range(NWARM):
        ps = setup_ps.tile([128, 512], F32, tag="sps", name=f"warm{w}")
        nc.tensor.matmul(ps[:], junk[:, 0:128], junk[:], start=True, stop=True)

    # ---- small constants ----
    onesrow = const.tile([1, 512], BF16)
    nc.vector.memset(onesrow[:], 1.0)
    neghalf = const.tile([1, 128], BF16)
    nc.vector.memset(neghalf[:], -0.5)
    ones64b = const.tile([64, 1], BF16)
    nc.vector.memset(ones64b[:], 1.0)
    ones64f = const.tile([64, 1], F32)
    nc.vector.memset(ones64f[:], 1.0)

    identb = const.tile([128, 128], BF16)
    make_identity(nc, identb[:])  # gpsimd
    # sliding reduction weights: Eb [128, 254], slice
    # [:, 126-2r : 254-2r] = ones at (p 0:64 -> row 2r), (p 64:128 -> 2r+1)
    Eb = const.tile([128, 254], BF16)
    nc.gpsimd.memset(Eb[:], 0.0)
    nc.gpsimd.memset(Eb[0:64, 126:127], 1.0)
    nc.gpsimd.memset(Eb[64:128, 127:128], 1.0)

    xTb = xallb[:, 0:N]
    xrTb = xallb[:, N : N + R]
    w_sb = xallb[:, N + R : N + R + F]
    bias_sb = xallb[0:1, N + R + F : N + R + 2 * F]

    # ---- setup: dbl (xwT stacked twice on partitions) via doubled W ----
    w2 = const.tile([128, 128], BF16)
    nc.vector.tensor_copy(w2[:, 0:F], w_sb)
    nc.vector.tensor_copy(w2[:, F : 2 * F], w_sb)
    dbl = const.tile([128, N], BF16)
    for h in range(2):
        ps = setup_ps.tile([128, 512], F32, tag="sps", name="sps")
        nc.tensor.matmul(
            ps[:], w2[:], xTb[:, 512 * h : 512 * h + 512], start=True, stop=True
        )
        nc.vector.tensor_copy(dbl[:, 512 * h : 512 * h + 512], ps[:])

    # ---- xwT_rows (exact i-side, f32) -> per-pair scalars S / negS ----
    xwT_rows = const.tile([64, R], F32)
    ps = setup_ps.tile([128, 512], F32, tag="sps", name="sps")
    nc.tensor.matmul(ps[0:64, 0:R], w_sb, xrTb[:], start=True, stop=True)
    nc.vector.tensor_copy(xwT_rows[:], ps[0:64, 0:R])

    S_bf = const.tile([128, NPAIR], F32)
    nc.vector.tensor_copy(S_bf[0:64, :], xwT_rows[:, 0:R:2])
    nc.vector.tensor_copy(S_bf[64:128, :], xwT_rows[:, 1:R:2])
    negS = const.tile([128, NPAIR], F32)
    nc.vector.tensor_scalar(negS[:], S_bf[:], -1.0, None, mybir.AluOpType.mult)

    # ---- s1row[j] = sum_f dbl[f,j] (bf16); s2row[i] = 0.5*(S2[i]+eps) ----
    s1row = const.tile([1, N], BF16)
    for h in range(2):
        ps = setup_ps.tile([128, 512], F32, tag="sps", name="sps")
        nc.tensor.matmul(
            ps[0:1, :], ones64b[:], dbl[0:64, 512 * h : 512 * h + 512],
            start=True, stop=True,
        )
        nc.scalar.copy(s1row[:, 512 * h : 512 * h + 512], ps[0:1, :])
    s2row = const.tile([1, R], BF16)
    ps = setup_ps.tile([128, 512], F32, tag="sps", name="sps")
    nc.tensor.matmul(ps[0:1, 0:R], ones64f[:], xwT_rows[:], start=True, stop=True)
    nc.scalar.activation(
        s2row[:], ps[0:1, 0:R], mybir.ActivationFunctionType.Copy,
        bias=0.5 * EPS, scale=0.5,
    )

    # ---- xw (bf16, j on partitions per chunk) + ones col -> final rhs ----
    xwb1 = const.tile([128, NCH * (F + 1)], BF16)
    for c in range(NCH):
        ps = setup_ps.tile([128, 512], F32, tag="sps", name="sps")
        nc.tensor.matmul(
            ps[:, 0:F], xTb[:, 128 * c : 128 * c + 128], w_sb, start=True, stop=True
        )
        nc.scalar.copy(xwb1[:, (F + 1) * c : (F + 1) * c + F], ps[:, 0:F])
    nc.vector.memset(xwb1[:, F : NCH * (F + 1) : F + 1], 1.0)

    xw_rows = const.tile([128, 2 * F], F32)
    for q in range(2):
        ps = setup_ps.tile([128, 512], F32, tag="sps", name="sps")
        nc.tensor.matmul(
            ps[:, 0:F], xrTb[:, 128 * q : 128 * q + 128], w_sb, start=True, stop=True
        )
        nc.vector.tensor_copy(xw_rows[:, F * q : F * q + F], ps[:, 0:F])

    # ---- bias broadcast [128, F] via K=1 matmul ----
    ones1 = const.tile([1, 128], BF16)
    nc.scalar.activation(
        ones1[:], xallb[0:1, 0:128], mybir.ActivationFunctionType.Copy,
        bias=1.0, scale=0.0,
    )
    bias_bc = const.tile([128, F], F32)
    ps = setup_ps.tile([128, 512], F32, tag="sps", name="sps")
    nc.tensor.matmul(ps[:, 0:F], ones1[:], bias_sb, start=True, stop=True)
    nc.vector.tensor_copy(bias_bc[:], ps[:, 0:F])

    modT = [
        const.tile([128, R], BF16, tag=f"modT{jc}", name=f"modT{jc}")
        for jc in range(NCH)
    ]
    out_sb = [const.tile([128, F], F32, tag=f"osb{q}", name=f"osb{q}") for q in range(2)]

    # ================= hot loop, software-pipelined across q =================
    slots = [_expand_sched() for _ in range(2)]
    adps = {}
    for q in range(2):
        adps[q] = [
            ad_ps.tile([128, 512], F32, tag=f"adps{k}", name=f"adps{q}_{k}")
            for k in range(2)
        ]

    def emit_v_producers(q, idxs):
        for s in (x for x in slots[q] if x["kind"] == "vb"):
            if s["t"] not in idxs:
                continue
            t = 64 * q + s["t"]
            tmpb = tmpv_pool.tile([128, N], BF16, tag="tv", name="tv")
            nc.vector.tensor_scalar(
                tmpb[:], dbl[:], S_bf[:, t : t + 1], 0.0,
                mybir.AluOpType.subtract, mybir.AluOpType.max,
            )
            s["tile"] = tmpb

    def emit_a_producers(q, idxs):
        for s in (x for x in slots[q] if x["kind"] == "ab"):
            if s["t"] not in idxs:
                continue
            t = 64 * q + s["t"]
            tmpa = tmp8a_pool.tile([128, N], BF16, tag="ta", name="ta")
            nc.scalar.activation(
                tmpa[:], dbl[:], mybir.ActivationFunctionType.Relu,
                bias=negS[:, t : t + 1], scale=1.0,
            )
            s["tile"] = tmpa

    def emit_pe_weave(q, lo, hi):
        """Emit sliding matmuls for weave positions [lo, hi) of block q.
        Position 0 opens the accumulation; the fold matmuls at the end
        close it (stop=True)."""
        order = _pe_order(slots[q])
        for i in range(lo, hi):
            s = order[i]
            r = s["t"]
            for k in range(2):
                nc.tensor.matmul(
                    adps[q][k][:],
                    Eb[:, 126 - 2 * r : 254 - 2 * r],
                    s["tile"][:, 512 * k : 512 * k + 512],
                    start=(i == 0),
                    stop=False,
                )

    def emit_pe_folds(q):
        # adps[q][k] += -0.5*S1[j]  and  += 0.5*(S2[i]+eps); closes group
        for k in range(2):
            nc.tensor.matmul(
                adps[q][k][:], neghalf[:], s1row[0:1, 512 * k : 512 * k + 512],
                start=False, stop=False,
            )
        for k in range(2):
            nc.tensor.matmul(
                adps[q][k][:], s2row[0:1, 128 * q : 128 * q + 128], onesrow[:],
                start=False, stop=True,
            )

    ctx_ep = {}

    def emit_recips(q):
        recipbf = recip_pool.tile([128, N], BF16, tag="recip", name="recip")
        for k in range(2):
            _act_recip(
                nc.scalar, recipbf[:, 512 * k : 512 * k + 512], adps[q][k][:],
                0.0, 2.0,
            )
        ctx_ep[q] = recipbf
        if "dbg_recip" in io:
            nc.sync.dma_start(io["dbg_recip"][128 * q : 128 * q + 128, :], recipbf[:])

    modbf_t = {}

    def emit_modbf_g(q):
        recipbf = ctx_ep[q]
        modbf = modbf_pool.tile([128, N], BF16, tag="modbf", name="modbf")
        modbf_t[q] = modbf
        nc.gpsimd.tensor_tensor(
            modbf[:, 0:512], adjq[q][:, 0:512], recipbf[:, 0:512],
            mybir.AluOpType.mult,
        )

    def emit_modbf_v(q):
        modbf = modbf_t[q]
        nc.vector.tensor_tensor(
            modbf[:, 512:N], adjq[q][:, 512:N], ctx_ep[q][:, 512:N],
            mybir.AluOpType.mult,
        )
        if "dbg_mod" in io:
            nc.sync.dma_start(io["dbg_mod"][128 * q : 128 * q + 128, :], modbf[:])

    def emit_transposes(q):
        modbf = modbf_t[q]
        for jc in range(NCH):
            tr = trfin_ps.tile([128, 128], BF16, tag="trfin", name="tr")
            nc.tensor.transpose(tr[:], modbf[:, 128 * jc : 128 * jc + 128], identb[:])
            if jc % 2 == 0:
                nc.vector.tensor_copy(modT[jc][:, 128 * q : 128 * q + 128], tr[:])
            else:
                nc.scalar.copy(modT[jc][:, 128 * q : 128 * q + 128], tr[:])

    fins = {}

    def emit_final_mm(q):
        fin = trfin_ps.tile([128, 128], F32, tag="trfin", name=f"fin{q}")
        fins[q] = fin
        for jc in range(NCH):
            nc.tensor.matmul(
                fin[:, 0 : F + 1],
                modT[jc][:, 128 * q : 128 * q + 128],
                xwb1[:, (F + 1) * jc : (F + 1) * jc + F + 1],
                start=(jc == 0),
                stop=(jc == NCH - 1),
            )

    def emit_combine(q):
        fin = fins[q]
        onemdeg = const.tile([128, 1], F32, tag=f"od{q}", name=f"od{q}")
        nc.vector.tensor_scalar(
            onemdeg[:], fin[:, F : F + 1], -1.0, 1.0,
            mybir.AluOpType.mult, mybir.AluOpType.add,
        )
        corr = const.tile([128, F], F32, tag=f"corr{q}", name=f"corr{q}")
        nc.vector.tensor_scalar(
            corr[:], xw_rows[:, F * q : F * q + F], onemdeg[:, 0:1], None,
            mybir.AluOpType.mult,
        )
        nc.vector.tensor_tensor(corr[:], corr[:], bias_bc[:], mybir.AluOpType.add)
        nc.vector.tensor_tensor(out_sb[q][:], corr[:], fin[:, 0:F], mybir.AluOpType.add)
        if "dbg_deg" in io:
            dsb = const.tile([128, 1], F32, tag=f"dsb{q}", name=f"dsb{q}")
            nc.vector.tensor_copy(dsb[:], fin[:, F : F + 1])
            nc.sync.dma_start(io["dbg_deg"][:, q : q + 1], dsb[:])
        nc.sync.dma_start(io["out_block"][128 * q : 128 * q + 128, :], out_sb[q][:])

    nab = sum(c for k, c in SCHED if k == "ab")
    nvb = sum(c for k, c in SCHED if k == "vb")
    vb_ts = [s["t"] for s in slots[0] if s["kind"] == "vb"]
    ab_ts = [s["t"] for s in slots[0] if s["kind"] == "ab"]

    # q0 producers + weave
    emit_v_producers(0, set(vb_ts))
    emit_a_producers(0, set(ab_ts))
    emit_pe_weave(0, 0, 64)
    emit_pe_folds(0)

    # q1 producers (leading chunk), then q0 epilogue interleaved with q1 weave
    emit_v_producers(1, set(vb_ts[:V_PRE]))
    emit_a_producers(1, set(ab_ts[:A_PRE]))
    emit_recips(0)  # ACT
    emit_modbf_g(0)  # gpsimd cols 0:512
    emit_modbf_v(0)  # V cols 512:1024
    emit_pe_weave(1, 0, PE_PRE1)
    emit_transposes(0)  # PE + gpsimd copies
    emit_pe_weave(1, PE_PRE1, PE_PRE1 + PE_PRE2)
    emit_final_mm(0)  # PE
    emit_v_producers(1, set(vb_ts[V_PRE:]))
    emit_a_producers(1, set(ab_ts[A_PRE:]))
    emit_pe_weave(1, PE_PRE1 + PE_PRE2, 64)
    emit_pe_folds(1)
    emit_combine(0)  # V + out DMA
    emit_recips(1)
    emit_modbf_g(1)
    emit_modbf_v(1)
    emit_transposes(1)
    emit_final_mm(1)
    emit_combine(1)


_CACHE = {}


def _build(debug=False):
    key = ("nc", debug)
    if key in _CACHE:
        return _CACHE[key]
    nc = bacc.Bacc()
    io = {
        "xallb": nc.declare_dram_parameter("xallb", [C, XALL_COLS], BF16, isOutput=False),
        "adjb": nc.declare_dram_parameter("adjb", [R, N], BF16, isOutput=False),
        "out_block": nc.declare_dram_parameter("out_block", [R, F], F32, isOutput=True),
    }
    if debug:
        io["dbg_recip"] = nc.declare_dram_parameter("dbg_recip", [R, N], BF16, isOutput=True)
        io["dbg_mod"] = nc.declare_dram_parameter("dbg_mod", [R, N], BF16, isOutput=True)
        io["dbg_deg"] = nc.declare_dram_parameter("dbg_deg", [128, 2], F32, isOutput=True)
    with tile.TileContext(nc) as tc:
        _body(tc, io)
    nc.finalize()
    _CACHE[key] = nc
    return nc


def _make_in_maps(x, adj, weight, bias):
    in_maps = []
    for core in range(8):
        b, blk = core // 4, core % 4
        r0 = blk * R
        xallb = np.zeros((C, XALL_COLS), dtype=ml_dtypes.bfloat16)
        xallb[:, 0:N] = x[b].T.astype(ml_dtypes.bfloat16)
        xallb[:, N : N + R] = x[b, r0 : r0 + R].T.astype(ml_dtypes.bfloat16)
        xallb[:, N + R : N + R + F] = weight.astype(ml_dtypes.bfloat16)
        xallb[0, N + R + F : N + R + 2 * F] = bias.astype(ml_dtypes.bfloat16)
        adjb = np.ascontiguousarray(adj[b, r0 : r0 + R]).copy()
        # Zero the self-edge: diag(mod_adj) cancels analytically in
        # out = (I - D + A~) xw, so drop it to avoid 1/0 on the diagonal.
        adjb[np.arange(R), r0 + np.arange(R)] = 0.0
        in_maps.append({"xallb": xallb, "adjb": adjb.astype(ml_dtypes.bfloat16)})
    return in_maps


def run(x, adj, weight, bias, trace=False, debug=False):
    nc = _build(debug=debug)
    res = run_bass_kernel_spmd(
        nc, _make_in_maps(x, adj, weight, bias), list(range(8)), trace=trace
    )
    out = np.empty((B, N, F), dtype=np.float32)
    for core in range(8):
        b, blk = core // 4, core % 4
        out[b, blk * R : blk * R + R] = res.results[core]["out_block"]
    return out, res


def kernel(x, adj, weight, bias):
    x = np.asarray(x, dtype=np.float32)
    adj = np.asarray(adj, dtype=np.float32)
    weight = np.asarray(weight, dtype=np.float32)
    bias = np.asarray(bias, dtype=np.float32)
    out, _ = run(x, adj, weight, bias, trace=False)
    return out


if __name__ == "__main__":
    pass


# revision 5
# speedup vs baseline: 1.1022x; 1.0437x over previous
"""DenseGTVConv Trainium2 kernel (v4).

out = (I - (D - A~)) @ (x @ W) + bias,  A~ = adj / clamp(pairwise_L1(xW), 1e-3)

Per i-pair, an elementwise relu(dbl - S) op feeds a PE partition-reduction
matmul (sliding-E), accumulating sum(relu) into PSUM.  The relu-identity
corrections -S1[j] and +S2[i]+eps are folded into the SAME PSUM
accumulation group via two K=1 matmuls, so the scalar engine computes
recip = 1/(2*psum) straight from PSUM.  modbf = adj * recip; transposed
chunks feed the final (A~ @ xw) matmul whose rhs carries an appended ones
column so deg falls out of the same matmul.

v4 structural points:
 - PE warmup matmuls on zeros during the startup window (HAM un-throttle)
 - xT DMA issued first (adj later) so dbl can start ~10us
 - S1/S2/eps folded into PSUM accumulation; recip reads PSUM directly
 - k-split accumulation close: the last KSPLIT pairs emit their k=0
   matmuls first so the k=0 half of the epilogue (recip, modbf, tr, fin)
   overlaps the k=1 sliding stream
 - cross-q software pipelining: q1 producers are emitted before q0's
   epilogue on each engine; PE interleaves q0 epilogue work into q1's
   sliding stream so it never idles
 - fused final combine via precomputed xwb_pre / negxw

Sharding: 8 cores = batch (2) x row-blocks (4 x 256 rows). Each core gets
the full x of its batch (needed on the j side), its 256-row slice of adj
(bf16, diag zeroed), and computes its 256-row slice of the output.

Self-contained: hardcoded shapes for B=2, N=1024, F_in=128, F_out=64.
"""
import sys

sys.path.insert(0, "/opt/trn_rl_repo")

from contextlib import ExitStack

import numpy as np
import ml_dtypes

import concourse.bass as bass
import concourse.bacc as bacc
import concourse.tile as tile
from concourse.masks import make_identity
from concourse import mybir
from concourse._compat import with_exitstack
from concourse.bass_utils import run_bass_kernel_spmd

F32 = mybir.dt.float32
BF16 = mybir.dt.bfloat16

B, N, C, F = 2, 1024, 128, 64  # batch, nodes, f_in, f_out
R = 256  # rows per core
NCH = N // 128  # 8 column chunks of 128
NPAIR = R // 2  # 128 i-pairs per core
EPS = 1e-3

# Packed bf16 setup input [128, 1408]:
#   cols    0:1024 : xT      (x_b.T)
#   cols 1024:1280 : xrT     (x_rows.T)
#   cols 1280:1344 : W       [128, 64]
#   cols 1344:1408 : bias in partition 0, cols 0:64
XALL_COLS = N + R + 2 * F

SCHED = [("ab", 20), ("vb", 44)]  # per q; 'ab' = ACT relu, 'vb' = vector relu

NWARM = 4  # PE warmup matmuls during the startup DMA window
KSPLIT = 12  # last pairs whose k0/k1 matmuls are split to close k=0 early
V_PRE = 10  # q1 vb tiles emitted before q0's epilogue V work
A_PRE = 10  # q1 ab tiles emitted before q0's recips
PE_PRE1 = 6  # q1 weave positions before q0's tr/fin (first half)
PE_PRE2 = 7  # q1 weave positions between q0 tr/fin halves


def _expand_sched():
    slots = []
    t = 0
    for kind, cnt in SCHED:
        for _ in range(cnt):
            slots.append(dict(kind=kind, t=t))
            t += 1
    assert t == 64, f"pairs = {t}"
    return slots


def _pe_order(slots):
    """Weave ab pairs through the vb stream so the PE always has ready work."""
    vb = [s for s in slots if s["kind"] == "vb"]
    ab = [s for s in slots if s["kind"] == "ab"]
    if not ab or not vb:
        return slots
    keyed = [((i + 0.5) / len(vb), s) for i, s in enumerate(vb)]
    keyed += [((j + 1.5) / (len(ab) + 1), s) for j, s in enumerate(ab)]
    return [s for _, s in sorted(keyed, key=lambda p: p[0])]


def _act_recip(sc, out, in_, bias, scale=1.0):
    """Scalar-engine Reciprocal(scale*in + bias), bypassing the accuracy
    guard.  Inputs here are in [35, 120] (pairwise L1 sums), far from the
    edge cases; the job tolerance is 2e-2 and the spline is ~1e-3-accurate."""
    inputs = [sc.lower_ap(in_)]
    for arg in (bias, scale, 0.0):  # bias, scale, alpha
        inputs.append(mybir.ImmediateValue(dtype=mybir.dt.float32, value=arg))
    return sc.add_instruction(
        mybir.InstActivation(
            name=sc.bass.get_next_instruction_name(),
            func=mybir.ActivationFunctionType.Reciprocal,
            ins=inputs,
            outs=[sc.lower_ap(out)],
        )
    )


@with_exitstack
def _body(ctx: ExitStack, tc: "tile.TileContext", io: dict):
    nc = tc.nc
    const = ctx.enter_context(tc.tile_pool(name="const", bufs=1))
    tmpv_pool = ctx.enter_context(tc.tile_pool(name="tmpv", bufs=14))
    tmp8a_pool = ctx.enter_context(tc.tile_pool(name="tmp8a", bufs=10))
    recip_pool = ctx.enter_context(tc.tile_pool(name="recip", bufs=2))
    modbf_pool = ctx.enter_context(tc.tile_pool(name="modbf", bufs=2))
    setup_ps = ctx.enter_context(tc.tile_pool(name="sps", bufs=2, space="PSUM"))
    ad_ps = ctx.enter_context(tc.tile_pool(name="adps", bufs=2, space="PSUM"))
    trfin_ps = ctx.enter_context(tc.tile_pool(name="trfin", bufs=2, space="PSUM"))

    # ---- tiles living in const pool ----
    xallb = const.tile([128, XALL_COLS], BF16)
    adjq = [
        const.tile([128, N], BF16, tag=f"adj{q}", name=f"adj{q}") for q in range(2)
    ]
    junk = const.tile([128, 512], BF16)

    # ---- V: junk memset first so PE warmup can start ASAP ----
    nc.vector.memset(junk[:], 0.0)

    # ---- input DMAs: xT first (critical path), adj later ----
    nc.scalar.dma_start(xallb[:, N:XALL_COLS], io["xallb"][:, N:XALL_COLS])
    nc.scalar.dma_start(xallb[:, 0:N], io["xallb"][:, 0:N])
    nc.gpsimd.dma_start(adjq[0][:], io["adjb"][0:128, :])
    nc.sync.dma_start(adjq[1][:], io["adjb"][128:256, :])

    # ---- PE warmup: zeros matmuls keep the PE busy from t~0 so the HAM
    # clock gate un-throttles before real work arrives ----
    for w in range(NWARM):
        ps = setup_ps.tile([128, 512], F32, tag="sps", name=f"warm{w}")
        nc.tensor.matmul(ps[:], junk[:, 0:128], junk[:], start=True, stop=True)

    # ---- small constants ----
    onesrow = const.tile([1, 512], BF16)
    nc.vector.memset(onesrow[:], 1.0)
    neghalf = const.tile([1, 128], BF16)
    nc.vector.memset(neghalf[:], -0.5)
    ones64b = const.tile([64, 1], BF16)
    nc.vector.memset(ones64b[:], 1.0)
    ones64f = const.tile([64, 1], F32)
    nc.vector.memset(ones64f[:], 1.0)

    identb = const.tile([128, 128], BF16)
    make_identity(nc, identb[:])  # gpsimd
    # sliding reduction weights: Eb [128, 254], slice
    # [:, 126-2r : 254-2r] = ones at (p 0:64 -> row 2r), (p 64:128 -> 2r+1)
    Eb = const.tile([128, 254], BF16)
    nc.gpsimd.memset(Eb[:], 0.0)
    nc.gpsimd.memset(Eb[0:64, 126:127], 1.0)
    nc.gpsimd.memset(Eb[64:128, 127:128], 1.0)

    xTb = xallb[:, 0:N]
    xrTb = xallb[:, N : N + R]
    w_sb = xallb[:, N + R : N + R + F]
    bias_sb = xallb[0:1, N + R + F : N + R + 2 * F]

    # ---- setup: dbl (xwT stacked twice on partitions) via doubled W ----
    w2 = const.tile([128, 128], BF16)
    nc.vector.tensor_copy(w2[:, 0:F], w_sb)
    nc.vector.tensor_copy(w2[:, F : 2 * F], w_sb)
    dbl = const.tile([128, N], BF16)
    for h in range(2):
        ps = setup_ps.tile([128, 512], F32, tag="sps", name="sps")
        nc.tensor.matmul(
            ps[:], w2[:], xTb[:, 512 * h : 512 * h + 512], start=True, stop=True
        )
        nc.vector.tensor_copy(dbl[:, 512 * h : 512 * h + 512], ps[:])

    # ---- xwT_rows (exact i-side, f32) -> per-pair scalars S / negS ----
    xwT_rows = const.tile([64, R], F32)
    ps = setup_ps.tile([128, 512], F32, tag="sps", name="sps")
    nc.tensor.matmul(ps[0:64, 0:R], w_sb, xrTb[:], start=True, stop=True)
    nc.vector.tensor_copy(xwT_rows[:], ps[0:64, 0:R])

    S_bf = const.tile([128, NPAIR], F32)
    nc.vector.tensor_copy(S_bf[0:64, :], xwT_rows[:, 0:R:2])
    nc.vector.tensor_copy(S_bf[64:128, :], xwT_rows[:, 1:R:2])
    negS = const.tile([128, NPAIR], F32)
    nc.vector.tensor_scalar(negS[:], S_bf[:], -1.0, None, mybir.AluOpType.mult)

    # ---- s1row[j] = sum_f dbl[f,j] (bf16); s2row[i] = 0.5*(S2[i]+eps) ----
    s1row = const.tile([1, N], BF16)
    for h in range(2):
        ps = setup_ps.tile([128, 512], F32, tag="sps", name="sps")
        nc.tensor.matmul(
            ps[0:1, :], ones64b[:], dbl[0:64, 512 * h : 512 * h + 512],
            start=True, stop=True,
        )
        nc.scalar.copy(s1row[:, 512 * h : 512 * h + 512], ps[0:1, :])
    s2row = const.tile([1, R], BF16)
    ps = setup_ps.tile([128, 512], F32, tag="sps", name="sps")
    nc.tensor.matmul(ps[0:1, 0:R], ones64f[:], xwT_rows[:], start=True, stop=True)
    nc.scalar.activation(
        s2row[:], ps[0:1, 0:R], mybir.ActivationFunctionType.Copy,
        bias=0.5 * EPS, scale=0.5,
    )

    # ---- xw (bf16, j on partitions per chunk) + ones col -> final rhs ----
    xwb1 = const.tile([128, NCH * (F + 1)], BF16)
    for c in range(NCH):
        ps = setup_ps.tile([128, 512], F32, tag="sps", name="sps")
        nc.tensor.matmul(
            ps[:, 0:F], xTb[:, 128 * c : 128 * c + 128], w_sb, start=True, stop=True
        )
        nc.scalar.copy(xwb1[:, (F + 1) * c : (F + 1) * c + F], ps[:, 0:F])
    nc.vector.memset(xwb1[:, F : NCH * (F + 1) : F + 1], 1.0)

    # xw_rows (f32) and fused-combine precomputes
    xw_rows = const.tile([128, 2 * F], F32)
    for q in range(2):
        ps = setup_ps.tile([128, 512], F32, tag="sps", name="sps")
        nc.tensor.matmul(
            ps[:, 0:F], xrTb[:, 128 * q : 128 * q + 128], w_sb, start=True, stop=True
        )
        nc.vector.tensor_copy(xw_rows[:, F * q : F * q + F], ps[:, 0:F])
    negxw = const.tile([128, 2 * F], F32)
    nc.vector.tensor_scalar(negxw[:], xw_rows[:], -1.0, None, mybir.AluOpType.mult)

    # ---- bias broadcast + xwb_pre = xw_rows + bias ----
    ones1 = const.tile([1, 128], BF16)
    nc.scalar.activation(
        ones1[:], xallb[0:1, 0:128], mybir.ActivationFunctionType.Copy,
        bias=1.0, scale=0.0,
    )
    xwb_pre = const.tile([128, 2 * F], F32)
    ps = setup_ps.tile([128, 512], F32, tag="sps", name="sps")
    nc.tensor.matmul(ps[:, 0:F], ones1[:], bias_sb, start=True, stop=True)
    for q in range(2):
        nc.vector.tensor_tensor(
            xwb_pre[:, F * q : F * q + F], xw_rows[:, F * q : F * q + F],
            ps[:, 0:F], mybir.AluOpType.add,
        )

    modT = [
        const.tile([128, R], BF16, tag=f"modT{jc}", name=f"modT{jc}")
        for jc in range(NCH)
    ]
    out_sb = [const.tile([128, F], F32, tag=f"osb{q}", name=f"osb{q}") for q in range(2)]

    # ================= hot loop, software-pipelined across q =================
    slots = [_expand_sched() for _ in range(2)]
    orders = [_pe_order(slots[q]) for q in range(2)]
    adps = {}
    for q in range(2):
        adps[q] = [
            ad_ps.tile([128, 512], F32, tag=f"adps{k}", name=f"adps{q}_{k}")
            for k in range(2)
        ]

    def emit_v_producers(q, idxs):
        for s in (x for x in slots[q] if x["kind"] == "vb"):
            if s["t"] not in idxs:
                continue
            t = 64 * q + s["t"]
            tmpb = tmpv_pool.tile([128, N], BF16, tag="tv", name="tv")
            nc.vector.tensor_scalar(
                tmpb[:], dbl[:], S_bf[:, t : t + 1], 0.0,
                mybir.AluOpType.subtract, mybir.AluOpType.max,
            )
            s["tile"] = tmpb

    def emit_a_producers(q, idxs):
        for s in (x for x in slots[q] if x["kind"] == "ab"):
            if s["t"] not in idxs:
                continue
            t = 64 * q + s["t"]
            tmpa = tmp8a_pool.tile([128, N], BF16, tag="ta", name="ta")
            nc.scalar.activation(
                tmpa[:], dbl[:], mybir.ActivationFunctionType.Relu,
                bias=negS[:, t : t + 1], scale=1.0,
            )
            s["tile"] = tmpa

    def emit_pe_weave(q, lo, hi, ks=(0, 1)):
        """Sliding matmuls for weave positions [lo, hi) of block q, k in ks."""
        order = orders[q]
        for i in range(lo, hi):
            s = order[i]
            r = s["t"]
            for k in ks:
                nc.tensor.matmul(
                    adps[q][k][:],
                    Eb[:, 126 - 2 * r : 254 - 2 * r],
                    s["tile"][:, 512 * k : 512 * k + 512],
                    start=(i == 0),
                    stop=False,
                )

    def emit_pe_folds(q, k):
        # adps[q][k] += -0.5*S1[j]  then  += 0.5*(S2[i]+eps); closes group
        nc.tensor.matmul(
            adps[q][k][:], neghalf[:], s1row[0:1, 512 * k : 512 * k + 512],
            start=False, stop=False,
        )
        nc.tensor.matmul(
            adps[q][k][:], s2row[0:1, 128 * q : 128 * q + 128], onesrow[:],
            start=False, stop=True,
        )

    recips = {}
    modbfs = {}

    def emit_recip(q, k):
        if q not in recips:
            recips[q] = recip_pool.tile([128, N], BF16, tag="recip", name="recip")
        _act_recip(
            nc.scalar, recips[q][:, 512 * k : 512 * k + 512], adps[q][k][:], 0.0, 2.0
        )
        if k == 1 and "dbg_recip" in io:
            nc.sync.dma_start(
                io["dbg_recip"][128 * q : 128 * q + 128, :], recips[q][:]
            )

    def emit_modbf(q, k, eng):
        if q not in modbfs:
            modbfs[q] = modbf_pool.tile([128, N], BF16, tag="modbf", name="modbf")
        sl = slice(512 * k, 512 * k + 512)
        eng.tensor_tensor(
            modbfs[q][:, sl], adjq[q][:, sl], recips[q][:, sl], mybir.AluOpType.mult
        )
        if k == 1 and "dbg_mod" in io:
            nc.sync.dma_start(io["dbg_mod"][128 * q : 128 * q + 128, :], modbfs[q][:])

    fins = {}

    def emit_trfin(q, jcs):
        """Transpose chunks jc and interleave the fin accumulation steps."""
        if q not in fins:
            fins[q] = trfin_ps.tile([128, 128], F32, tag="trfin", name=f"fin{q}")
        fin = fins[q]
        pend = []
        for jc in jcs:
            tr = trfin_ps.tile([128, 128], BF16, tag="trfin", name="tr")
            nc.tensor.transpose(
                tr[:], modbfs[q][:, 128 * jc : 128 * jc + 128], identb[:]
            )
            if jc % 2 == 0:
                nc.vector.tensor_copy(modT[jc][:, 128 * q : 128 * q + 128], tr[:])
            else:
                nc.scalar.copy(modT[jc][:, 128 * q : 128 * q + 128], tr[:])
            pend.append(jc)
            if len(pend) >= 2:
                _fin_step(q, fin, pend.pop(0))
        for jc in pend:
            _fin_step(q, fin, jc)

    def _fin_step(q, fin, jc):
        nc.tensor.matmul(
            fin[:, 0 : F + 1],
            modT[jc][:, 128 * q : 128 * q + 128],
            xwb1[:, (F + 1) * jc : (F + 1) * jc + F + 1],
            start=(jc == 0),
            stop=(jc == NCH - 1),
        )

    def emit_combine(q):
        fin = fins[q]
        # out = (xw_rows + bias) + fin - deg*xw_rows
        corr = const.tile([128, F], F32, tag=f"corr{q}", name=f"corr{q}")
        nc.vector.tensor_scalar(
            corr[:], negxw[:, F * q : F * q + F], fin[:, F : F + 1], None,
            mybir.AluOpType.mult,
        )
        s = const.tile([128, F], F32, tag=f"s{q}", name=f"s{q}")
        nc.vector.tensor_tensor(
            s[:], xwb_pre[:, F * q : F * q + F], fin[:, 0:F], mybir.AluOpType.add
        )
        nc.vector.tensor_tensor(out_sb[q][:], s[:], corr[:], mybir.AluOpType.add)
        if "dbg_deg" in io:
            dsb = const.tile([128, 1], F32, tag=f"dsb{q}", name=f"dsb{q}")
            nc.vector.tensor_copy(dsb[:], fin[:, F : F + 1])
            nc.sync.dma_start(io["dbg_deg"][:, q : q + 1], dsb[:])
        nc.sync.dma_start(io["out_block"][128 * q : 128 * q + 128, :], out_sb[q][:])

    vb_ts = [s["t"] for s in slots[0] if s["kind"] == "vb"]
    ab_ts = [s["t"] for s in slots[0] if s["kind"] == "ab"]
    NW = 64
    SPLIT = NW - KSPLIT

    # ---- q0 producers + weave (k-split close) ----
    emit_v_producers(0, set(vb_ts))
    emit_a_producers(0, set(ab_ts))
    emit_pe_weave(0, 0, SPLIT)
    emit_pe_weave(0, SPLIT, NW, ks=(0,))
    emit_pe_folds(0, 0)
    emit_recip(0, 0)  # ACT, overlaps k1 stream
    emit_modbf(0, 0, nc.gpsimd)
    emit_pe_weave(0, SPLIT, NW, ks=(1,))
    emit_pe_folds(0, 1)

    # ---- q0 epilogue interleaved with q1 stream ----
    emit_v_producers(1, set(vb_ts[:V_PRE]))
    emit_a_producers(1, set(ab_ts[:A_PRE]))
    emit_recip(0, 1)  # ACT
    emit_modbf(0, 1, nc.vector)
    emit_pe_weave(1, 0, PE_PRE1)
    emit_trfin(0, [0, 1, 2, 3])
    emit_pe_weave(1, PE_PRE1, PE_PRE1 + PE_PRE2)
    emit_trfin(0, [4, 5, 6, 7])
    emit_v_producers(1, set(vb_ts[V_PRE:]))
    emit_a_producers(1, set(ab_ts[A_PRE:]))
    emit_pe_weave(1, PE_PRE1 + PE_PRE2, SPLIT)
    emit_combine(0)  # V + out DMA
    emit_pe_weave(1, SPLIT, NW, ks=(0,))
    emit_pe_folds(1, 0)
    emit_recip(1, 0)  # ACT
    emit_modbf(1, 0, nc.gpsimd)
    emit_pe_weave(1, SPLIT, SPLIT + 6, ks=(1,))
    emit_trfin(1, [0, 1])
    emit_pe_weave(1, SPLIT + 6, NW, ks=(1,))
    emit_trfin(1, [2, 3])
    emit_pe_folds(1, 1)
    emit_recip(1, 1)  # ACT
    emit_modbf(1, 1, nc.vector)
    emit_trfin(1, [4, 5, 6, 7])
    emit_combine(1)


_CACHE = {}


def _build(debug=False):
    key = ("nc", debug)
    if key in _CACHE:
        return _CACHE[key]
    nc = bacc.Bacc()
    io = {
        "xallb": nc.declare_dram_parameter("xallb", [C, XALL_COLS], BF16, isOutput=False),
        "adjb": nc.declare_dram_parameter("adjb", [R, N], BF16, isOutput=False),
        "out_block": nc.declare_dram_parameter("out_block", [R, F], F32, isOutput=True),
    }
    if debug:
        io["dbg_recip"] = nc.declare_dram_parameter("dbg_recip", [R, N], BF16, isOutput=True)
        io["dbg_mod"] = nc.declare_dram_parameter("dbg_mod", [R, N], BF16, isOutput=True)
        io["dbg_deg"] = nc.declare_dram_parameter("dbg_deg", [128, 2], F32, isOutput=True)
    with tile.TileContext(nc) as tc:
        _body(tc, io)
    nc.finalize()
    _CACHE[key] = nc
    return nc


def _make_in_maps(x, adj, weight, bias):
    in_maps = []
    for core in range(8):
        b, blk = core // 4, core % 4
        r0 = blk * R
        xallb = np.zeros((C, XALL_COLS), dtype=ml_dtypes.bfloat16)
        xallb[:, 0:N] = x[b].T.astype(ml_dtypes.bfloat16)
        xallb[:, N : N + R] = x[b, r0 : r0 + R].T.astype(ml_dtypes.bfloat16)
        xallb[:, N + R : N + R + F] = weight.astype(ml_dtypes.bfloat16)
        xallb[0, N + R + F : N + R + 2 * F] = bias.astype(ml_dtypes.bfloat16)
        adjb = np.ascontiguousarray(adj[b, r0 : r0 + R]).copy()
        # Zero the self-edge: diag(mod_adj) cancels analytically in
        # out = (I - D + A~) xw, so drop it to avoid 1/0 on the diagonal.
        adjb[np.arange(R), r0 + np.arange(R)] = 0.0
        in_maps.append({"xallb": xallb, "adjb": adjb.astype(ml_dtypes.bfloat16)})
    return in_maps


def run(x, adj, weight, bias, trace=False, debug=False):
    nc = _build(debug=debug)
    res = run_bass_kernel_spmd(
        nc, _make_in_maps(x, adj, weight, bias), list(range(8)), trace=trace
    )
    out = np.empty((B, N, F), dtype=np.float32)
    for core in range(8):
        b, blk = core // 4, core % 4
        out[b, blk * R : blk * R + R] = res.results[core]["out_block"]
    return out, res


def kernel(x, adj, weight, bias):
    x = np.asarray(x, dtype=np.float32)
    adj = np.asarray(adj, dtype=np.float32)
    weight = np.asarray(weight, dtype=np.float32)
    bias = np.asarray(bias, dtype=np.float32)
    out, _ = run(x, adj, weight, bias, trace=False)
    return out


# revision 8
# speedup vs baseline: 1.1753x; 1.0663x over previous
"""DenseGTVConv Trainium2 kernel (v5).

out = (I - (D - A~)) @ (x @ W) + bias,  A~ = adj / clamp(pairwise_L1(xW), 1e-3)

Per i-pair, an elementwise relu(dbl - S) op feeds a PE partition-reduction
matmul (sliding-E), accumulating sum(relu) into PSUM.  The relu-identity
corrections -S1[j] and +S2[i]+eps are folded into the SAME PSUM
accumulation group via two K=1 matmuls, so the scalar engine computes
recip = 1/(2*psum) straight from PSUM.  modbf = adj * recip; transposed
chunks feed the final (A~ @ xw) matmul whose rhs carries an appended ones
column so deg falls out of the same matmul.

v5: ACT-produced pairs move to fp8 DoubleRow duos.  A duo packs pairs
(d, d+32) as two contiguous [128,1024] fp8 relu tiles; the DoubleRow
matmul (lhsT [p][2][128], rhs [p][2][512] 3D APs) reduces both pairs in
one N=512 stream — ~1.8x PE throughput for those pairs.  The duo weight
pattern is FIXED (anchors at cols 62/63 + 254/255 of a sliding window
offset 62-2d), verified by hardware probe.

Plus: PE warmup matmuls during the startup window (HAM un-throttle),
xT DMA split across two queues ahead of adj, k-split accumulation close
(epilogue k=0 half overlaps the k=1 stream), cross-q software pipelining,
fused final combine.

Sharding: 8 cores = batch (2) x row-blocks (4 x 256 rows). Each core gets
the full x of its batch (needed on the j side), its 256-row slice of adj
(bf16, diag zeroed), and computes its 256-row slice of the output.

Self-contained: hardcoded shapes for B=2, N=1024, F_in=128, F_out=64.
"""
import sys

sys.path.insert(0, "/opt/trn_rl_repo")

from contextlib import ExitStack

import numpy as np
import ml_dtypes

import concourse.bass as bass
import concourse.bacc as bacc
import concourse.tile as tile
from concourse.masks import make_identity
from concourse import mybir
from concourse._compat import with_exitstack
from concourse.bass_utils import run_bass_kernel_spmd

F32 = mybir.dt.float32
BF16 = mybir.dt.bfloat16
FP8 = mybir.dt.float8e4

B, N, C, F = 2, 1024, 128, 64  # batch, nodes, f_in, f_out
R = 256  # rows per core
NCH = N // 128  # 8 column chunks of 128
NPAIR = R // 2  # 128 i-pairs per core
EPS = 1e-3

# Packed bf16 setup input [128, 1408]:
#   cols    0:1024 : xT      (x_b.T)
#   cols 1024:1280 : xrT     (x_rows.T)
#   cols 1280:1344 : W       [128, 64]
#   cols 1344:1408 : bias in partition 0, cols 0:64
XALL_COLS = N + R + 2 * F

ND8 = 11  # fp8 duos per q (each covers pairs d and d+32), produced on ACT
NVB = 64 - 2 * ND8  # bf16 vector pairs per q

NWARM = 5  # PE warmup matmuls during the startup DMA window
KSPLIT = 16  # trailing weave slots whose k0/k1 are split to close k=0 early
V_PRE = 10  # q1 vb tiles emitted before q0's epilogue V work
A_PRE = 3  # q1 a8 duos emitted before q0's recips
PE_PRE1 = 6  # q1 weave positions before q0's tr/fin (first half)
PE_PRE2 = 7  # q1 weave positions between q0 tr/fin halves


def _expand_sched():
    """Slots: 'a8' duos d=0..ND8-1 (pairs d, d+32) + 'vb' the remaining t."""
    slots = [dict(kind="a8", d=d) for d in range(ND8)]
    used = set(range(ND8)) | set(range(32, 32 + ND8))
    slots += [dict(kind="vb", t=t) for t in range(64) if t not in used]
    assert len(slots) == ND8 + NVB
    return slots


def _pe_order(slots):
    """Weave a8 duos through the vb stream so the PE always has ready work."""
    vb = [s for s in slots if s["kind"] == "vb"]
    a8 = [s for s in slots if s["kind"] == "a8"]
    if not a8 or not vb:
        return slots
    keyed = [((i + 0.5) / len(vb), s) for i, s in enumerate(vb)]
    keyed += [((j + 1.5) / (len(a8) + 1), s) for j, s in enumerate(a8)]
    return [s for _, s in sorted(keyed, key=lambda p: p[0])]


def _act_recip(sc, out, in_, bias, scale=1.0):
    """Scalar-engine Reciprocal(scale*in + bias), bypassing the accuracy
    guard.  Inputs here are in [35, 120] (pairwise L1 sums), far from the
    edge cases; the job tolerance is 2e-2 and the spline is ~1e-3-accurate."""
    inputs = [sc.lower_ap(in_)]
    for arg in (bias, scale, 0.0):  # bias, scale, alpha
        inputs.append(mybir.ImmediateValue(dtype=mybir.dt.float32, value=arg))
    return sc.add_instruction(
        mybir.InstActivation(
            name=sc.bass.get_next_instruction_name(),
            func=mybir.ActivationFunctionType.Reciprocal,
            ins=inputs,
            outs=[sc.lower_ap(out)],
        )
    )


@with_exitstack
def _body(ctx: ExitStack, tc: "tile.TileContext", io: dict):
    nc = tc.nc
    const = ctx.enter_context(tc.tile_pool(name="const", bufs=1))
    tmpv_pool = ctx.enter_context(tc.tile_pool(name="tmpv", bufs=14))
    tmp8a_pool = ctx.enter_context(tc.tile_pool(name="tmp8a", bufs=5))
    recip_pool = ctx.enter_context(tc.tile_pool(name="recip", bufs=2))
    modbf_pool = ctx.enter_context(tc.tile_pool(name="modbf", bufs=2))
    setup_ps = ctx.enter_context(tc.tile_pool(name="sps", bufs=2, space="PSUM"))
    ad_ps = ctx.enter_context(tc.tile_pool(name="adps", bufs=2, space="PSUM"))
    trfin_ps = ctx.enter_context(tc.tile_pool(name="trfin", bufs=2, space="PSUM"))

    # ---- tiles living in const pool ----
    xallb = const.tile([128, XALL_COLS], BF16)
    adjq = [
        const.tile([128, N], BF16, tag=f"adj{q}", name=f"adj{q}") for q in range(2)
    ]
    junk = const.tile([128, 512], BF16)

    # ---- V: junk memset first so PE warmup can start ASAP ----
    nc.vector.memset(junk[:], 0.0)

    # ---- input DMAs: xT split across two queues first, adj later ----
    nc.scalar.dma_start(xallb[:, N:XALL_COLS], io["xallb"][:, N:XALL_COLS])
    nc.scalar.dma_start(xallb[:, 0:512], io["xallb"][:, 0:512])
    nc.sync.dma_start(xallb[:, 512:N], io["xallb"][:, 512:N])
    nc.gpsimd.dma_start(adjq[0][:], io["adjb"][0:128, :])
    nc.sync.dma_start(adjq[1][:], io["adjb"][128:256, :])

    # ---- PE warmup: zeros matmuls keep the PE busy from t~0 so the HAM
    # clock gate un-throttles before real work arrives ----
    for w in range(NWARM):
        ps = setup_ps.tile([128, 512], F32, tag="sps", name=f"warm{w}")
        nc.tensor.matmul(ps[:], junk[:, 0:128], junk[:], start=True, stop=True)

    # ---- small constants ----
    onesrow = const.tile([1, 512], BF16)
    nc.vector.memset(onesrow[:], 1.0)
    neghalf = const.tile([1, 128], BF16)
    nc.vector.memset(neghalf[:], -0.5)
    ones64b = const.tile([64, 1], BF16)
    nc.vector.memset(ones64b[:], 1.0)
    ones64f = const.tile([64, 1], F32)
    nc.vector.memset(ones64f[:], 1.0)

    identb = const.tile([128, 128], BF16)
    make_identity(nc, identb[:])  # gpsimd
    # bf16 sliding weights: Eb [128, 254], slice [:, 126-2r : 254-2r]
    Eb = const.tile([128, 254], BF16)
    nc.gpsimd.memset(Eb[:], 0.0)
    nc.gpsimd.memset(Eb[0:64, 126:127], 1.0)
    nc.gpsimd.memset(Eb[64:128, 127:128], 1.0)
    # fp8 DoubleRow duo weights: slice [:, 62-2d : 62-2d+256]; anchors fixed
    # at cols 62/63 (pair d -> rows 2d,2d+1) and 254/255 (pair d+32).
    Eb8 = const.tile([128, 320], FP8)
    nc.gpsimd.memset(Eb8[:], 0.0)
    nc.gpsimd.memset(Eb8[0:64, 62:63], 1.0)
    nc.gpsimd.memset(Eb8[64:128, 63:64], 1.0)
    nc.gpsimd.memset(Eb8[0:64, 254:255], 1.0)
    nc.gpsimd.memset(Eb8[64:128, 255:256], 1.0)

    xTb = xallb[:, 0:N]
    xrTb = xallb[:, N : N + R]
    w_sb = xallb[:, N + R : N + R + F]
    bias_sb = xallb[0:1, N + R + F : N + R + 2 * F]

    # ---- setup: dbl (xwT stacked twice on partitions) via doubled W ----
    w2 = const.tile([128, 128], BF16)
    nc.vector.tensor_copy(w2[:, 0:F], w_sb)
    nc.vector.tensor_copy(w2[:, F : 2 * F], w_sb)
    dbl = const.tile([128, N], BF16)
    for h in range(2):
        ps = setup_ps.tile([128, 512], F32, tag="sps", name="sps")
        nc.tensor.matmul(
            ps[:], w2[:], xTb[:, 512 * h : 512 * h + 512], start=True, stop=True
        )
        nc.vector.tensor_copy(dbl[:, 512 * h : 512 * h + 512], ps[:])

    # ---- xwT_rows (exact i-side, f32) -> per-pair scalars S / negS ----
    xwT_rows = const.tile([64, R], F32)
    ps = setup_ps.tile([128, 512], F32, tag="sps", name="sps")
    nc.tensor.matmul(ps[0:64, 0:R], w_sb, xrTb[:], start=True, stop=True)
    nc.vector.tensor_copy(xwT_rows[:], ps[0:64, 0:R])

    S_bf = const.tile([128, NPAIR], F32)
    nc.vector.tensor_copy(S_bf[0:64, :], xwT_rows[:, 0:R:2])
    nc.vector.tensor_copy(S_bf[64:128, :], xwT_rows[:, 1:R:2])
    negS = const.tile([128, NPAIR], F32)
    nc.vector.tensor_scalar(negS[:], S_bf[:], -1.0, None, mybir.AluOpType.mult)

    # ---- s1row[j] = sum_f dbl[f,j] (bf16); s2row[i] = 0.5*(S2[i]+eps) ----
    s1row = const.tile([1, N], BF16)
    for h in range(2):
        ps = setup_ps.tile([128, 512], F32, tag="sps", name="sps")
        nc.tensor.matmul(
            ps[0:1, :], ones64b[:], dbl[0:64, 512 * h : 512 * h + 512],
            start=True, stop=True,
        )
        nc.scalar.copy(s1row[:, 512 * h : 512 * h + 512], ps[0:1, :])
    s2row = const.tile([1, R], BF16)
    ps = setup_ps.tile([128, 512], F32, tag="sps", name="sps")
    nc.tensor.matmul(ps[0:1, 0:R], ones64f[:], xwT_rows[:], start=True, stop=True)
    nc.scalar.activation(
        s2row[:], ps[0:1, 0:R], mybir.ActivationFunctionType.Copy,
        bias=0.5 * EPS, scale=0.5,
    )

    # ---- xw (bf16, j on partitions per chunk) + ones col -> final rhs ----
    # 8 matmuls packed into one PSUM tile, one strided copy out.
    xwb1 = const.tile([128, NCH * (F + 1)], BF16)
    ps_xw = setup_ps.tile([128, 512], F32, tag="sps", name="psxw")
    for c in range(NCH):
        nc.tensor.matmul(
            ps_xw[:, F * c : F * c + F], xTb[:, 128 * c : 128 * c + 128], w_sb,
            start=True, stop=True,
        )
    nc.scalar.copy(
        xwb1[:].rearrange("p (c f) -> p c f", c=NCH)[:, :, 0:F],
        ps_xw[:].rearrange("p (c f) -> p c f", c=NCH),
    )
    nc.vector.memset(xwb1[:, F : NCH * (F + 1) : F + 1], 1.0)

    # xw_rows (f32) and fused-combine precomputes
    xw_rows = const.tile([128, 2 * F], F32)
    ps = setup_ps.tile([128, 512], F32, tag="sps", name="sps")
    for q in range(2):
        nc.tensor.matmul(
            ps[:, F * q : F * q + F], xrTb[:, 128 * q : 128 * q + 128], w_sb,
            start=True, stop=True,
        )
    nc.vector.tensor_copy(xw_rows[:], ps[:, 0 : 2 * F])
    negxw = const.tile([128, 2 * F], F32)
    nc.vector.tensor_scalar(negxw[:], xw_rows[:], -1.0, None, mybir.AluOpType.mult)

    # ---- bias broadcast + xwb_pre = xw_rows + bias ----
    ones1 = const.tile([1, 128], BF16)
    nc.scalar.activation(
        ones1[:], xallb[0:1, 0:128], mybir.ActivationFunctionType.Copy,
        bias=1.0, scale=0.0,
    )
    xwb_pre = const.tile([128, 2 * F], F32)
    ps = setup_ps.tile([128, 512], F32, tag="sps", name="sps")
    nc.tensor.matmul(ps[:, 0:F], ones1[:], bias_sb, start=True, stop=True)
    for q in range(2):
        nc.vector.tensor_tensor(
            xwb_pre[:, F * q : F * q + F], xw_rows[:, F * q : F * q + F],
            ps[:, 0:F], mybir.AluOpType.add,
        )

    modT = [
        const.tile([128, R], BF16, tag=f"modT{jc}", name=f"modT{jc}")
        for jc in range(NCH)
    ]
    out_sb = [const.tile([128, F], F32, tag=f"osb{q}", name=f"osb{q}") for q in range(2)]

    # ================= hot loop, software-pipelined across q =================
    slots = [_expand_sched() for _ in range(2)]
    orders = [_pe_order(slots[q]) for q in range(2)]
    adps = {}
    for q in range(2):
        adps[q] = [
            ad_ps.tile([128, 512], F32, tag=f"adps{k}", name=f"adps{q}_{k}")
            for k in range(2)
        ]

    def emit_v_producers(q, idxs):
        for j, s in enumerate(x for x in slots[q] if x["kind"] == "vb"):
            if j not in idxs:
                continue
            t = 64 * q + s["t"]
            tmpb = tmpv_pool.tile([128, N], BF16, tag="tv", name="tv")
            nc.vector.tensor_scalar(
                tmpb[:], dbl[:], S_bf[:, t : t + 1], 0.0,
                mybir.AluOpType.subtract, mybir.AluOpType.max,
            )
            s["tile"] = tmpb

    def emit_a_producers(q, idxs):
        for j, s in enumerate(x for x in slots[q] if x["kind"] == "a8"):
            if j not in idxs:
                continue
            d = s["d"]
            duo = tmp8a_pool.tile([128, 2 * N], FP8, tag="ta", name="ta")
            for half, t in ((0, 64 * q + d), (1, 64 * q + d + 32)):
                nc.scalar.activation(
                    duo[:, N * half : N * half + N], dbl[:],
                    mybir.ActivationFunctionType.Relu,
                    bias=negS[:, t : t + 1], scale=1.0,
                )
            s["tile"] = duo

    def emit_pe_weave(q, lo, hi, ks=(0, 1)):
        """Sliding matmuls for weave positions [lo, hi) of block q, k in ks."""
        order = orders[q]
        for i in range(lo, hi):
            s = order[i]
            for k in ks:
                if s["kind"] == "vb":
                    r = s["t"]
                    nc.tensor.matmul(
                        adps[q][k][:],
                        Eb[:, 126 - 2 * r : 254 - 2 * r],
                        s["tile"][:, 512 * k : 512 * k + 512],
                        start=(i == 0),
                        stop=False,
                    )
                else:
                    d = s["d"]
                    lhsT = Eb8[:, 62 - 2 * d : 62 - 2 * d + 256].rearrange(
                        "p (s m) -> p s m", s=2
                    )
                    rhs = s["tile"][:].rearrange("p (s n) -> p s n", s=2)[
                        :, :, 512 * k : 512 * k + 512
                    ]
                    nc.tensor.matmul(
                        adps[q][k][:], lhsT, rhs,
                        start=(i == 0), stop=False,
                        perf_mode=mybir.MatmulPerfMode.DoubleRow,
                    )

    def emit_pe_folds(q, k):
        # adps[q][k] += -0.5*S1[j]  then  += 0.5*(S2[i]+eps); closes group
        nc.tensor.matmul(
            adps[q][k][:], neghalf[:], s1row[0:1, 512 * k : 512 * k + 512],
            start=False, stop=False,
        )
        nc.tensor.matmul(
            adps[q][k][:], s2row[0:1, 128 * q : 128 * q + 128], onesrow[:],
            start=False, stop=True,
        )

    recips = {}
    modbfs = {}

    def emit_recip(q, k):
        if q not in recips:
            recips[q] = recip_pool.tile([128, N], BF16, tag="recip", name="recip")
        _act_recip(
            nc.scalar, recips[q][:, 512 * k : 512 * k + 512], adps[q][k][:], 0.0, 2.0
        )
        if k == 1 and "dbg_recip" in io:
            nc.sync.dma_start(
                io["dbg_recip"][128 * q : 128 * q + 128, :], recips[q][:]
            )

    def emit_modbf(q, k, eng):
        if q not in modbfs:
            modbfs[q] = modbf_pool.tile([128, N], BF16, tag="modbf", name="modbf")
        sl = slice(512 * k, 512 * k + 512)
        eng.tensor_tensor(
            modbfs[q][:, sl], adjq[q][:, sl], recips[q][:, sl], mybir.AluOpType.mult
        )
        if k == 1 and "dbg_mod" in io:
            nc.sync.dma_start(io["dbg_mod"][128 * q : 128 * q + 128, :], modbfs[q][:])

    fins = {}

    def emit_trfin(q, jcs):
        """Transpose chunks jc and interleave the fin accumulation steps."""
        if q not in fins:
            fins[q] = trfin_ps.tile([128, 128], F32, tag="trfin", name=f"fin{q}")
        fin = fins[q]
        pend = []
        for jc in jcs:
            tr = trfin_ps.tile([128, 128], BF16, tag="trfin", name="tr")
            nc.tensor.transpose(
                tr[:], modbfs[q][:, 128 * jc : 128 * jc + 128], identb[:]
            )
            if jc % 2 == 0:
                nc.vector.tensor_copy(modT[jc][:, 128 * q : 128 * q + 128], tr[:])
            else:
                nc.scalar.copy(modT[jc][:, 128 * q : 128 * q + 128], tr[:])
            pend.append(jc)
            if len(pend) >= 2:
                _fin_step(q, fin, pend.pop(0))
        for jc in pend:
            _fin_step(q, fin, jc)

    def _fin_step(q, fin, jc):
        nc.tensor.matmul(
            fin[:, 0 : F + 1],
            modT[jc][:, 128 * q : 128 * q + 128],
            xwb1[:, (F + 1) * jc : (F + 1) * jc + F + 1],
            start=(jc == 0),
            stop=(jc == NCH - 1),
        )

    def emit_combine(q):
        fin = fins[q]
        # out = (xw_rows + bias) + fin - deg*xw_rows
        corr = const.tile([128, F], F32, tag=f"corr{q}", name=f"corr{q}")
        nc.vector.tensor_scalar(
            corr[:], negxw[:, F * q : F * q + F], fin[:, F : F + 1], None,
            mybir.AluOpType.mult,
        )
        s = const.tile([128, F], F32, tag=f"s{q}", name=f"s{q}")
        nc.vector.tensor_tensor(
            s[:], xwb_pre[:, F * q : F * q + F], fin[:, 0:F], mybir.AluOpType.add
        )
        nc.vector.tensor_tensor(out_sb[q][:], s[:], corr[:], mybir.AluOpType.add)
        if "dbg_deg" in io:
            dsb = const.tile([128, 1], F32, tag=f"dsb{q}", name=f"dsb{q}")
            nc.vector.tensor_copy(dsb[:], fin[:, F : F + 1])
            nc.sync.dma_start(io["dbg_deg"][:, q : q + 1], dsb[:])
        nc.sync.dma_start(io["out_block"][128 * q : 128 * q + 128, :], out_sb[q][:])

    NW = ND8 + NVB  # weave length
    SPLIT = NW - KSPLIT

    # producer indices needed for the q1 weave prefix emitted before the
    # q0 epilogue: count kinds in orders[1][0:PE_PRE1+PE_PRE2]
    prefix = orders[1][0 : PE_PRE1 + PE_PRE2]
    n_vb_pre = max(V_PRE, sum(1 for s in prefix if s["kind"] == "vb") + 1)
    n_a8_pre = max(A_PRE, sum(1 for s in prefix if s["kind"] == "a8") + 1)
    n_vb_pre = min(n_vb_pre, NVB)
    n_a8_pre = min(n_a8_pre, ND8)

    # ---- q0 producers + weave (k-split close) ----
    emit_v_producers(0, set(range(NVB)))
    emit_a_producers(0, set(range(ND8)))
    emit_pe_weave(0, 0, SPLIT)
    emit_pe_weave(0, SPLIT, NW, ks=(0,))
    emit_pe_folds(0, 0)
    emit_recip(0, 0)  # ACT, overlaps k1 stream
    emit_modbf(0, 0, nc.gpsimd)
    emit_pe_weave(0, SPLIT, NW, ks=(1,))
    emit_pe_folds(0, 1)

    # ---- q0 epilogue interleaved with q1 stream ----
    emit_v_producers(1, set(range(n_vb_pre)))
    emit_a_producers(1, set(range(n_a8_pre)))
    emit_recip(0, 1)  # ACT
    emit_modbf(0, 1, nc.vector)
    emit_pe_weave(1, 0, PE_PRE1)
    emit_trfin(0, [0, 1, 2, 3])
    emit_pe_weave(1, PE_PRE1, PE_PRE1 + PE_PRE2)
    emit_trfin(0, [4, 5, 6, 7])
    emit_v_producers(1, set(range(n_vb_pre, NVB)))
    emit_a_producers(1, set(range(n_a8_pre, ND8)))
    emit_pe_weave(1, PE_PRE1 + PE_PRE2, SPLIT)
    emit_combine(0)  # V + out DMA
    emit_pe_weave(1, SPLIT, NW, ks=(0,))
    emit_pe_folds(1, 0)
    emit_recip(1, 0)  # ACT
    emit_modbf(1, 0, nc.vector)
    emit_pe_weave(1, SPLIT, SPLIT + 4, ks=(1,))
    emit_trfin(1, [0])
    emit_pe_weave(1, SPLIT + 4, SPLIT + 8, ks=(1,))
    emit_trfin(1, [1])
    emit_pe_weave(1, SPLIT + 8, SPLIT + 12, ks=(1,))
    emit_trfin(1, [2, 3])
    emit_pe_weave(1, SPLIT + 12, NW, ks=(1,))
    emit_pe_folds(1, 1)
    emit_recip(1, 1)  # ACT
    emit_modbf(1, 1, nc.vector)
    emit_trfin(1, [4, 5, 6, 7])
    emit_combine(1)


_CACHE = {}


def _build(debug=False):
    key = ("nc", debug)
    if key in _CACHE:
        return _CACHE[key]
    nc = bacc.Bacc()
    io = {
        "xallb": nc.declare_dram_parameter("xallb", [C, XALL_COLS], BF16, isOutput=False),
        "adjb": nc.declare_dram_parameter("adjb", [R, N], BF16, isOutput=False),
        "out_block": nc.declare_dram_parameter("out_block", [R, F], F32, isOutput=True),
    }
    if debug:
        io["dbg_recip"] = nc.declare_dram_parameter("dbg_recip", [R, N], BF16, isOutput=True)
        io["dbg_mod"] = nc.declare_dram_parameter("dbg_mod", [R, N], BF16, isOutput=True)
        io["dbg_deg"] = nc.declare_dram_parameter("dbg_deg", [128, 2], F32, isOutput=True)
    with tile.TileContext(nc) as tc:
        _body(tc, io)
    nc.finalize()
    _CACHE[key] = nc
    return nc


def _make_in_maps(x, adj, weight, bias):
    in_maps = []
    for core in range(8):
        b, blk = core // 4, core % 4
        r0 = blk * R
        xallb = np.zeros((C, XALL_COLS), dtype=ml_dtypes.bfloat16)
        xallb[:, 0:N] = x[b].T.astype(ml_dtypes.bfloat16)
        xallb[:, N : N + R] = x[b, r0 : r0 + R].T.astype(ml_dtypes.bfloat16)
        xallb[:, N + R : N + R + F] = weight.astype(ml_dtypes.bfloat16)
        xallb[0, N + R + F : N + R + 2 * F] = bias.astype(ml_dtypes.bfloat16)
        adjb = np.ascontiguousarray(adj[b, r0 : r0 + R]).copy()
        # Zero the self-edge: diag(mod_adj) cancels analytically in
        # out = (I - D + A~) xw, so drop it to avoid 1/0 on the diagonal.
        adjb[np.arange(R), r0 + np.arange(R)] = 0.0
        in_maps.append({"xallb": xallb, "adjb": adjb.astype(ml_dtypes.bfloat16)})
    return in_maps


def run(x, adj, weight, bias, trace=False, debug=False):
    nc = _build(debug=debug)
    res = run_bass_kernel_spmd(
        nc, _make_in_maps(x, adj, weight, bias), list(range(8)), trace=trace
    )
    out = np.empty((B, N, F), dtype=np.float32)
    for core in range(8):
        b, blk = core // 4, core % 4
        out[b, blk * R : blk * R + R] = res.results[core]["out_block"]
    return out, res


def kernel(x, adj, weight, bias):
    x = np.asarray(x, dtype=np.float32)
    adj = np.asarray(adj, dtype=np.float32)
    weight = np.asarray(weight, dtype=np.float32)
    bias = np.asarray(bias, dtype=np.float32)
    out, _ = run(x, adj, weight, bias, trace=False)
    return out


# revision 9
# speedup vs baseline: 1.2208x; 1.0387x over previous
"""DenseGTVConv Trainium2 kernel (v5).

out = (I - (D - A~)) @ (x @ W) + bias,  A~ = adj / clamp(pairwise_L1(xW), 1e-3)

Per i-pair, an elementwise relu(dbl - S) op feeds a PE partition-reduction
matmul (sliding-E), accumulating sum(relu) into PSUM.  The relu-identity
corrections -S1[j] and +S2[i]+eps are folded into the SAME PSUM
accumulation group via two K=1 matmuls, so the scalar engine computes
recip = 1/(2*psum) straight from PSUM.  modbf = adj * recip; transposed
chunks feed the final (A~ @ xw) matmul whose rhs carries an appended ones
column so deg falls out of the same matmul.

v5: ACT-produced pairs move to fp8 DoubleRow duos.  A duo packs pairs
(d, d+32) as two contiguous [128,1024] fp8 relu tiles; the DoubleRow
matmul (lhsT [p][2][128], rhs [p][2][512] 3D APs) reduces both pairs in
one N=512 stream — ~1.8x PE throughput for those pairs.  The duo weight
pattern is FIXED (anchors at cols 62/63 + 254/255 of a sliding window
offset 62-2d), verified by hardware probe.

Plus: PE warmup matmuls during the startup window (HAM un-throttle),
xT DMA split across two queues ahead of adj, k-split accumulation close
(epilogue k=0 half overlaps the k=1 stream), cross-q software pipelining,
fused final combine.

Sharding: 8 cores = batch (2) x row-blocks (4 x 256 rows). Each core gets
the full x of its batch (needed on the j side), its 256-row slice of adj
(bf16, diag zeroed), and computes its 256-row slice of the output.

Self-contained: hardcoded shapes for B=2, N=1024, F_in=128, F_out=64.
"""
import sys

sys.path.insert(0, "/opt/trn_rl_repo")

from contextlib import ExitStack

import numpy as np
import ml_dtypes

import concourse.bass as bass
import concourse.bacc as bacc
import concourse.tile as tile
from concourse.masks import make_identity
from concourse import mybir
from concourse._compat import with_exitstack
from concourse.bass_utils import run_bass_kernel_spmd

F32 = mybir.dt.float32
BF16 = mybir.dt.bfloat16
FP8 = mybir.dt.float8e4

B, N, C, F = 2, 1024, 128, 64  # batch, nodes, f_in, f_out
R = 256  # rows per core
NCH = N // 128  # 8 column chunks of 128
NPAIR = R // 2  # 128 i-pairs per core
EPS = 1e-3

# Packed bf16 setup input [128, 1408]:
#   cols    0:1024 : xT      (x_b.T)
#   cols 1024:1280 : xrT     (x_rows.T)
#   cols 1280:1344 : W       [128, 64]
#   cols 1344:1408 : bias in partition 0, cols 0:64
XALL_COLS = N + R + 2 * F

ND8 = 10  # fp8 duos per q (each covers pairs d and d+32), produced on ACT
NVB = 64 - 2 * ND8  # bf16 vector pairs per q

NWARM = 5  # PE warmup matmuls during the startup DMA window
KSPLIT = 16  # trailing weave slots whose k0/k1 are split to close k=0 early
V_PRE = 16  # q1 vb tiles emitted before q0's epilogue V work
A_PRE = 3  # q1 a8 duos emitted before q0's recips
PE_PRE1 = 6  # q1 weave positions before q0's tr/fin (first half)
PE_PRE2 = 7  # q1 weave positions between q0 tr/fin halves


def _expand_sched():
    """Slots: 'a8' duos d=0..ND8-1 (pairs d, d+32) + 'vb' the remaining t."""
    slots = [dict(kind="a8", d=d) for d in range(ND8)]
    used = set(range(ND8)) | set(range(32, 32 + ND8))
    slots += [dict(kind="vb", t=t) for t in range(64) if t not in used]
    assert len(slots) == ND8 + NVB
    return slots


def _pe_order(slots):
    """Weave a8 duos through the vb stream so the PE always has ready work."""
    vb = [s for s in slots if s["kind"] == "vb"]
    a8 = [s for s in slots if s["kind"] == "a8"]
    if not a8 or not vb:
        return slots
    keyed = [((i + 0.5) / len(vb), s) for i, s in enumerate(vb)]
    keyed += [((j + 1.5) / (len(a8) + 1), s) for j, s in enumerate(a8)]
    return [s for _, s in sorted(keyed, key=lambda p: p[0])]


def _act_recip(sc, out, in_, bias, scale=1.0):
    """Scalar-engine Reciprocal(scale*in + bias), bypassing the accuracy
    guard.  Inputs here are in [35, 120] (pairwise L1 sums), far from the
    edge cases; the job tolerance is 2e-2 and the spline is ~1e-3-accurate."""
    inputs = [sc.lower_ap(in_)]
    for arg in (bias, scale, 0.0):  # bias, scale, alpha
        inputs.append(mybir.ImmediateValue(dtype=mybir.dt.float32, value=arg))
    return sc.add_instruction(
        mybir.InstActivation(
            name=sc.bass.get_next_instruction_name(),
            func=mybir.ActivationFunctionType.Reciprocal,
            ins=inputs,
            outs=[sc.lower_ap(out)],
        )
    )


@with_exitstack
def _body(ctx: ExitStack, tc: "tile.TileContext", io: dict):
    nc = tc.nc
    const = ctx.enter_context(tc.tile_pool(name="const", bufs=1))
    tmpv_pool = ctx.enter_context(tc.tile_pool(name="tmpv", bufs=14))
    tmp8a_pool = ctx.enter_context(tc.tile_pool(name="tmp8a", bufs=5))
    recip_pool = ctx.enter_context(tc.tile_pool(name="recip", bufs=2))
    modbf_pool = ctx.enter_context(tc.tile_pool(name="modbf", bufs=2))
    setup_ps = ctx.enter_context(tc.tile_pool(name="sps", bufs=2, space="PSUM"))
    ad_ps = ctx.enter_context(tc.tile_pool(name="adps", bufs=2, space="PSUM"))
    trfin_ps = ctx.enter_context(tc.tile_pool(name="trfin", bufs=3, space="PSUM"))

    # ---- tiles living in const pool ----
    xallb = const.tile([128, XALL_COLS], BF16)
    adjq = [
        const.tile([128, N], BF16, tag=f"adj{q}", name=f"adj{q}") for q in range(2)
    ]
    junk = const.tile([128, 512], BF16)

    # ---- V: junk memset first so PE warmup can start ASAP ----
    nc.vector.memset(junk[:], 0.0)

    # ---- input DMAs: xT split across two queues first, adj later ----
    nc.scalar.dma_start(xallb[:, N:XALL_COLS], io["xallb"][:, N:XALL_COLS])
    nc.scalar.dma_start(xallb[:, 0:512], io["xallb"][:, 0:512])
    nc.sync.dma_start(xallb[:, 512:N], io["xallb"][:, 512:N])
    nc.gpsimd.dma_start(adjq[0][:], io["adjb"][0:128, :])
    nc.sync.dma_start(adjq[1][:], io["adjb"][128:256, :])

    # ---- PE warmup: zeros matmuls keep the PE busy from t~0 so the HAM
    # clock gate un-throttles before real work arrives ----
    for w in range(NWARM):
        ps = setup_ps.tile([128, 512], F32, tag="sps", name=f"warm{w}")
        nc.tensor.matmul(ps[:], junk[:, 0:128], junk[:], start=True, stop=True)

    # ---- small constants ----
    onesrow = const.tile([1, 512], BF16)
    nc.vector.memset(onesrow[:], 1.0)
    neghalf = const.tile([1, 128], BF16)
    nc.vector.memset(neghalf[:], -0.5)
    ones64b = const.tile([64, 1], BF16)
    nc.vector.memset(ones64b[:], 1.0)
    ones64f = const.tile([64, 1], F32)
    nc.vector.memset(ones64f[:], 1.0)

    identb = const.tile([128, 128], BF16)
    make_identity(nc, identb[:])  # gpsimd
    # bf16 sliding weights: Eb [128, 254], slice [:, 126-2r : 254-2r]
    Eb = const.tile([128, 254], BF16)
    nc.gpsimd.memset(Eb[:], 0.0)
    nc.gpsimd.memset(Eb[0:64, 126:127], 1.0)
    nc.gpsimd.memset(Eb[64:128, 127:128], 1.0)
    # fp8 DoubleRow duo weights: slice [:, 62-2d : 62-2d+256]; anchors fixed
    # at cols 62/63 (pair d -> rows 2d,2d+1) and 254/255 (pair d+32).
    Eb8 = const.tile([128, 320], FP8)
    nc.gpsimd.memset(Eb8[:], 0.0)
    nc.gpsimd.memset(Eb8[0:64, 62:63], 1.0)
    nc.gpsimd.memset(Eb8[64:128, 63:64], 1.0)
    nc.gpsimd.memset(Eb8[0:64, 254:255], 1.0)
    nc.gpsimd.memset(Eb8[64:128, 255:256], 1.0)

    xTb = xallb[:, 0:N]
    xrTb = xallb[:, N : N + R]
    w_sb = xallb[:, N + R : N + R + F]
    bias_sb = xallb[0:1, N + R + F : N + R + 2 * F]

    # ---- setup: dbl (xwT stacked twice on partitions) via doubled W ----
    w2 = const.tile([128, 128], BF16)
    nc.vector.tensor_copy(w2[:, 0:F], w_sb)
    nc.vector.tensor_copy(w2[:, F : 2 * F], w_sb)
    dbl = const.tile([128, N], BF16)
    for h in range(2):
        ps = setup_ps.tile([128, 512], F32, tag="sps", name="sps")
        nc.tensor.matmul(
            ps[:], w2[:], xTb[:, 512 * h : 512 * h + 512], start=True, stop=True
        )
        nc.vector.tensor_copy(dbl[:, 512 * h : 512 * h + 512], ps[:])

    # ---- xwT_rows (exact i-side, f32) -> per-pair scalars S / negS ----
    xwT_rows = const.tile([64, R], F32)
    ps = setup_ps.tile([128, 512], F32, tag="sps", name="sps")
    nc.tensor.matmul(ps[0:64, 0:R], w_sb, xrTb[:], start=True, stop=True)
    nc.vector.tensor_copy(xwT_rows[:], ps[0:64, 0:R])

    S_bf = const.tile([128, NPAIR], F32)
    nc.vector.tensor_copy(S_bf[0:64, :], xwT_rows[:, 0:R:2])
    nc.vector.tensor_copy(S_bf[64:128, :], xwT_rows[:, 1:R:2])
    negS = const.tile([128, NPAIR], F32)
    nc.vector.tensor_scalar(negS[:], S_bf[:], -1.0, None, mybir.AluOpType.mult)

    # ---- s1row[j] = sum_f dbl[f,j] (bf16); s2row[i] = 0.5*(S2[i]+eps) ----
    s1row = const.tile([1, N], BF16)
    for h in range(2):
        ps = setup_ps.tile([128, 512], F32, tag="sps", name="sps")
        nc.tensor.matmul(
            ps[0:1, :], ones64b[:], dbl[0:64, 512 * h : 512 * h + 512],
            start=True, stop=True,
        )
        nc.scalar.copy(s1row[:, 512 * h : 512 * h + 512], ps[0:1, :])
    s2row = const.tile([1, R], BF16)
    ps = setup_ps.tile([128, 512], F32, tag="sps", name="sps")
    nc.tensor.matmul(ps[0:1, 0:R], ones64f[:], xwT_rows[:], start=True, stop=True)
    nc.scalar.activation(
        s2row[:], ps[0:1, 0:R], mybir.ActivationFunctionType.Copy,
        bias=0.5 * EPS, scale=0.5,
    )

    # ---- xw (bf16, j on partitions per chunk) + ones col -> final rhs ----
    # 8 matmuls packed into one PSUM tile, one strided copy out.
    xwb1 = const.tile([128, NCH * (F + 1)], BF16)
    ps_xw = setup_ps.tile([128, 512], F32, tag="sps", name="psxw")
    for c in range(NCH):
        nc.tensor.matmul(
            ps_xw[:, F * c : F * c + F], xTb[:, 128 * c : 128 * c + 128], w_sb,
            start=True, stop=True,
        )
    nc.scalar.copy(
        xwb1[:].rearrange("p (c f) -> p c f", c=NCH)[:, :, 0:F],
        ps_xw[:].rearrange("p (c f) -> p c f", c=NCH),
    )
    nc.vector.memset(xwb1[:, F : NCH * (F + 1) : F + 1], 1.0)

    # xw_rows (f32) and fused-combine precomputes
    xw_rows = const.tile([128, 2 * F], F32)
    ps = setup_ps.tile([128, 512], F32, tag="sps", name="sps")
    for q in range(2):
        nc.tensor.matmul(
            ps[:, F * q : F * q + F], xrTb[:, 128 * q : 128 * q + 128], w_sb,
            start=True, stop=True,
        )
    nc.vector.tensor_copy(xw_rows[:], ps[:, 0 : 2 * F])
    negxw = const.tile([128, 2 * F], F32)
    nc.vector.tensor_scalar(negxw[:], xw_rows[:], -1.0, None, mybir.AluOpType.mult)

    # ---- bias broadcast + xwb_pre = xw_rows + bias ----
    ones1 = const.tile([1, 128], BF16)
    nc.scalar.activation(
        ones1[:], xallb[0:1, 0:128], mybir.ActivationFunctionType.Copy,
        bias=1.0, scale=0.0,
    )
    xwb_pre = const.tile([128, 2 * F], F32)
    ps = setup_ps.tile([128, 512], F32, tag="sps", name="sps")
    nc.tensor.matmul(ps[:, 0:F], ones1[:], bias_sb, start=True, stop=True)
    for q in range(2):
        nc.vector.tensor_tensor(
            xwb_pre[:, F * q : F * q + F], xw_rows[:, F * q : F * q + F],
            ps[:, 0:F], mybir.AluOpType.add,
        )

    modT = [
        const.tile([128, R], BF16, tag=f"modT{jc}", name=f"modT{jc}")
        for jc in range(NCH)
    ]
    out_sb = [const.tile([128, F], F32, tag=f"osb{q}", name=f"osb{q}") for q in range(2)]

    # ================= hot loop, software-pipelined across q =================
    slots = [_expand_sched() for _ in range(2)]
    orders = [_pe_order(slots[q]) for q in range(2)]
    adps = {}
    for q in range(2):
        adps[q] = [
            ad_ps.tile(
                [128, 512], F32, tag=f"adps{k}", name=f"adps{q}_{k}",
                bufs=(1 if k == 0 else 2),
            )
            for k in range(2)
        ]

    def emit_v_producers(q, idxs):
        for j, s in enumerate(x for x in slots[q] if x["kind"] == "vb"):
            if j not in idxs:
                continue
            t = 64 * q + s["t"]
            tmpb = tmpv_pool.tile([128, N], BF16, tag="tv", name="tv")
            nc.vector.tensor_scalar(
                tmpb[:], dbl[:], S_bf[:, t : t + 1], 0.0,
                mybir.AluOpType.subtract, mybir.AluOpType.max,
            )
            s["tile"] = tmpb

    def emit_a_producers(q, idxs):
        for j, s in enumerate(x for x in slots[q] if x["kind"] == "a8"):
            if j not in idxs:
                continue
            d = s["d"]
            duo = tmp8a_pool.tile([128, 2 * N], FP8, tag="ta", name="ta")
            for half, t in ((0, 64 * q + d), (1, 64 * q + d + 32)):
                nc.scalar.activation(
                    duo[:, N * half : N * half + N], dbl[:],
                    mybir.ActivationFunctionType.Relu,
                    bias=negS[:, t : t + 1], scale=1.0,
                )
            s["tile"] = duo

    def emit_pe_weave(q, lo, hi, ks=(0, 1)):
        """Sliding matmuls for weave positions [lo, hi) of block q, k in ks."""
        order = orders[q]
        for i in range(lo, hi):
            s = order[i]
            for k in ks:
                if s["kind"] == "vb":
                    r = s["t"]
                    nc.tensor.matmul(
                        adps[q][k][:],
                        Eb[:, 126 - 2 * r : 254 - 2 * r],
                        s["tile"][:, 512 * k : 512 * k + 512],
                        start=(i == 0),
                        stop=False,
                    )
                else:
                    d = s["d"]
                    lhsT = Eb8[:, 62 - 2 * d : 62 - 2 * d + 256].rearrange(
                        "p (s m) -> p s m", s=2
                    )
                    rhs = s["tile"][:].rearrange("p (s n) -> p s n", s=2)[
                        :, :, 512 * k : 512 * k + 512
                    ]
                    nc.tensor.matmul(
                        adps[q][k][:], lhsT, rhs,
                        start=(i == 0), stop=False,
                        perf_mode=mybir.MatmulPerfMode.DoubleRow,
                    )

    def emit_pe_folds(q, k):
        # adps[q][k] += -0.5*S1[j]  then  += 0.5*(S2[i]+eps); closes group
        nc.tensor.matmul(
            adps[q][k][:], neghalf[:], s1row[0:1, 512 * k : 512 * k + 512],
            start=False, stop=False,
        )
        nc.tensor.matmul(
            adps[q][k][:], s2row[0:1, 128 * q : 128 * q + 128], onesrow[:],
            start=False, stop=True,
        )

    recips = {}
    modbfs = {}

    def emit_recip(q, k):
        if q not in recips:
            recips[q] = recip_pool.tile([128, N], BF16, tag="recip", name="recip")
        _act_recip(
            nc.scalar, recips[q][:, 512 * k : 512 * k + 512], adps[q][k][:], 0.0, 2.0
        )
        if k == 1 and "dbg_recip" in io:
            nc.sync.dma_start(
                io["dbg_recip"][128 * q : 128 * q + 128, :], recips[q][:]
            )

    def emit_modbf(q, k, eng):
        if q not in modbfs:
            modbfs[q] = modbf_pool.tile([128, N], BF16, tag="modbf", name="modbf")
        sl = slice(512 * k, 512 * k + 512)
        eng.tensor_tensor(
            modbfs[q][:, sl], adjq[q][:, sl], recips[q][:, sl], mybir.AluOpType.mult
        )
        if k == 1 and "dbg_mod" in io:
            nc.sync.dma_start(io["dbg_mod"][128 * q : 128 * q + 128, :], modbfs[q][:])

    fins = {}

    def emit_trfin(q, jcs):
        """Transpose chunks jc and interleave the fin accumulation steps."""
        if q not in fins:
            fins[q] = trfin_ps.tile([128, 128], F32, tag="trfin", name=f"fin{q}")
        fin = fins[q]
        pend = []
        for jc in jcs:
            tr = trfin_ps.tile([128, 128], BF16, tag="trfin", name="tr")
            nc.tensor.transpose(
                tr[:], modbfs[q][:, 128 * jc : 128 * jc + 128], identb[:]
            )
            if jc % 2 == 0:
                nc.vector.tensor_copy(modT[jc][:, 128 * q : 128 * q + 128], tr[:])
            else:
                nc.scalar.copy(modT[jc][:, 128 * q : 128 * q + 128], tr[:])
            pend.append(jc)
            if len(pend) >= 2:
                _fin_step(q, fin, pend.pop(0))
        for jc in pend:
            _fin_step(q, fin, jc)

    def _fin_step(q, fin, jc):
        nc.tensor.matmul(
            fin[:, 0 : F + 1],
            modT[jc][:, 128 * q : 128 * q + 128],
            xwb1[:, (F + 1) * jc : (F + 1) * jc + F + 1],
            start=(jc == 0),
            stop=(jc == NCH - 1),
        )

    def emit_combine(q):
        fin = fins[q]
        # out = (xw_rows + bias) + fin - deg*xw_rows
        corr = const.tile([128, F], F32, tag=f"corr{q}", name=f"corr{q}")
        nc.vector.tensor_scalar(
            corr[:], negxw[:, F * q : F * q + F], fin[:, F : F + 1], None,
            mybir.AluOpType.mult,
        )
        s = const.tile([128, F], F32, tag=f"s{q}", name=f"s{q}")
        nc.vector.tensor_tensor(
            s[:], xwb_pre[:, F * q : F * q + F], fin[:, 0:F], mybir.AluOpType.add
        )
        nc.vector.tensor_tensor(out_sb[q][:], s[:], corr[:], mybir.AluOpType.add)
        if "dbg_deg" in io:
            dsb = const.tile([128, 1], F32, tag=f"dsb{q}", name=f"dsb{q}")
            nc.vector.tensor_copy(dsb[:], fin[:, F : F + 1])
            nc.sync.dma_start(io["dbg_deg"][:, q : q + 1], dsb[:])
        nc.sync.dma_start(io["out_block"][128 * q : 128 * q + 128, :], out_sb[q][:])

    NW = ND8 + NVB  # weave length
    SPLIT = NW - KSPLIT

    # producer indices needed for the q1 weave prefix emitted before the
    # q0 epilogue: count kinds in orders[1][0:PE_PRE1+PE_PRE2]
    prefix = orders[1][0 : PE_PRE1 + PE_PRE2]
    n_vb_pre = max(V_PRE, sum(1 for s in prefix if s["kind"] == "vb") + 4)
    n_a8_pre = max(A_PRE, sum(1 for s in prefix if s["kind"] == "a8") + 1)
    n_vb_pre = min(n_vb_pre, NVB)
    n_a8_pre = min(n_a8_pre, ND8)

    # ---- q0 producers + weave (k-split close) ----
    emit_v_producers(0, set(range(NVB)))
    emit_a_producers(0, set(range(ND8)))
    emit_pe_weave(0, 0, SPLIT)
    emit_pe_weave(0, SPLIT, NW, ks=(0,))
    emit_pe_folds(0, 0)
    emit_recip(0, 0)  # ACT, overlaps k1 stream
    emit_modbf(0, 0, nc.gpsimd)
    emit_pe_weave(0, SPLIT, NW, ks=(1,))
    emit_pe_folds(0, 1)

    # ---- q0 epilogue interleaved with q1 stream ----
    emit_v_producers(1, set(range(n_vb_pre)))
    emit_a_producers(1, set(range(1)))
    emit_recip(0, 1)  # ACT right after its fold; only 1 q1 duo ahead of it
    emit_modbf(0, 1, nc.gpsimd)
    emit_a_producers(1, set(range(1, n_a8_pre)))
    emit_pe_weave(1, 0, PE_PRE1)
    emit_trfin(0, [0, 1, 2, 3])
    emit_pe_weave(1, PE_PRE1, PE_PRE1 + PE_PRE2)
    emit_trfin(0, [4, 5, 6, 7])
    emit_v_producers(1, set(range(n_vb_pre, NVB)))
    emit_a_producers(1, set(range(n_a8_pre, ND8)))
    emit_pe_weave(1, PE_PRE1 + PE_PRE2, SPLIT)
    emit_combine(0)  # V + out DMA
    emit_pe_weave(1, SPLIT, NW, ks=(0,))
    emit_pe_folds(1, 0)
    emit_recip(1, 0)  # ACT
    emit_modbf(1, 0, nc.vector)
    emit_pe_weave(1, SPLIT, SPLIT + 4, ks=(1,))
    emit_trfin(1, [0])
    emit_pe_weave(1, SPLIT + 4, SPLIT + 8, ks=(1,))
    emit_trfin(1, [1])
    emit_pe_weave(1, SPLIT + 8, SPLIT + 12, ks=(1,))
    emit_trfin(1, [2, 3])
    emit_pe_weave(1, SPLIT + 12, NW, ks=(1,))
    emit_pe_folds(1, 1)
    emit_recip(1, 1)  # ACT
    emit_modbf(1, 1, nc.vector)
    emit_trfin(1, [4, 5, 6, 7])
    emit_combine(1)


_CACHE = {}


def _build(debug=False):
    key = ("nc", debug)
    if key in _CACHE:
        return _CACHE[key]
    nc = bacc.Bacc()
    io = {
        "xallb": nc.declare_dram_parameter("xallb", [C, XALL_COLS], BF16, isOutput=False),
        "adjb": nc.declare_dram_parameter("adjb", [R, N], BF16, isOutput=False),
        "out_block": nc.declare_dram_parameter("out_block", [R, F], F32, isOutput=True),
    }
    if debug:
        io["dbg_recip"] = nc.declare_dram_parameter("dbg_recip", [R, N], BF16, isOutput=True)
        io["dbg_mod"] = nc.declare_dram_parameter("dbg_mod", [R, N], BF16, isOutput=True)
        io["dbg_deg"] = nc.declare_dram_parameter("dbg_deg", [128, 2], F32, isOutput=True)
    with tile.TileContext(nc) as tc:
        _body(tc, io)
    nc.finalize()
    _CACHE[key] = nc
    return nc


def _make_in_maps(x, adj, weight, bias):
    in_maps = []
    for core in range(8):
        b, blk = core // 4, core % 4
        r0 = blk * R
        xallb = np.zeros((C, XALL_COLS), dtype=ml_dtypes.bfloat16)
        xallb[:, 0:N] = x[b].T.astype(ml_dtypes.bfloat16)
        xallb[:, N : N + R] = x[b, r0 : r0 + R].T.astype(ml_dtypes.bfloat16)
        xallb[:, N + R : N + R + F] = weight.astype(ml_dtypes.bfloat16)
        xallb[0, N + R + F : N + R + 2 * F] = bias.astype(ml_dtypes.bfloat16)
        adjb = np.ascontiguousarray(adj[b, r0 : r0 + R]).copy()
        # Zero the self-edge: diag(mod_adj) cancels analytically in
        # out = (I - D + A~) xw, so drop it to avoid 1/0 on the diagonal.
        adjb[np.arange(R), r0 + np.arange(R)] = 0.0
        in_maps.append({"xallb": xallb, "adjb": adjb.astype(ml_dtypes.bfloat16)})
    return in_maps


def run(x, adj, weight, bias, trace=False, debug=False):
    nc = _build(debug=debug)
    res = run_bass_kernel_spmd(
        nc, _make_in_maps(x, adj, weight, bias), list(range(8)), trace=trace
    )
    out = np.empty((B, N, F), dtype=np.float32)
    for core in range(8):
        b, blk = core // 4, core % 4
        out[b, blk * R : blk * R + R] = res.results[core]["out_block"]
    return out, res


def kernel(x, adj, weight, bias):
    x = np.asarray(x, dtype=np.float32)
    adj = np.asarray(adj, dtype=np.float32)
    weight = np.asarray(weight, dtype=np.float32)
    bias = np.asarray(bias, dtype=np.float32)
    out, _ = run(x, adj, weight, bias, trace=False)
    return out
